# revision 42
# baseline (speedup 1.0000x reference)
"""Trainium2 Bass kernel for the nn_Points problem.

Renders N=1024 anisotropic "diamond" points onto a 3x256x384 canvas:
    t = (pixel - loc) @ M_n          (2-vector per pixel per point)
    mapped = relu(1 - (|t0|+|t1|)/2)
    canvas = sigmoid(4 * sum_n mapped * color_n)

Strategy (8 NeuronCores, full inputs in / full output out):
  * Spatial-shard the canvas: core c renders rows [32c, 32c+32).
  * Within a core: 24 spatial tiles of 4 rows x 128 cols (512 px).
  * Host-side exact culling: point n can touch a tile only if
    sigma_min(M_n) * dist2(loc_n, tile_rect) <= 2  (else |t|_1 >= 2
    everywhere in the tile and mapped is exactly 0).  Measured <= ~82
    points per tile, so one 128-slot point tile per spatial tile.
  * u = t0+t1, v = t0-t1 are affine in (gy, gx) -> computed as K=8
    fp16 matmuls (hi/lo split of coords/consts for fp32-grade accuracy):
        out[pt, px] = W[k, pt].T @ G[k, px]
    The u and v matmuls run CONCURRENTLY on different PE row-groups
    (K=8 uses 8 of 128 array rows; v's operands are replicated at
    partitions 32..39 so its matmul lands on row-group 1 and starts
    ~5ns after u's -- measured ~2x on the uv stream).
  * |u|,|v|: one ACT Abs over both PSUM banks (hardware allows only ONE
    PSUM operand per DVE/ACT instruction and walrus has no float-abs DVE
    ALU op, so ACT's Abs is the only cheap |.| fold); then DVE
    tensor_tensor max and a fused tensor_scalar m'' = min(d,2)-2
    (= -2*relu(1-d/2)); the -0.5 sign/scale is folded into the colors.
  * canvas: matmul with mapped (fp16, SBUF) as the stationary operand,
    colors [128pts, 3] as moving operand; accumulates [128px, 3] blocks
    into one persistent PSUM bank laid out [128, 32rows*3blk*3ch].
  * Input DMAs are split into per-chunk-range pieces (W/G interleaved)
    so the first chunks' operands land ~6us earlier than a monolithic
    transfer would allow; compute starts while later pieces stream in.
  * One sigmoid(4x) ACT over the whole core's canvas + one DMA out.

Measured on 8 axon trn2 cores: 45.3us (baseline 48.8us).  Steady state
is ~1.16us/chunk with ACT (Abs) ~96% and PE ~96% busy simultaneously;
~7us NEFF/engine-init preamble, ~1.5us first-DMA latency and a ~4us
multi-core drain/barrier tail are framework-fixed.  Experiments that
did NOT help on this silicon (kept behind env flags, all default-off):
fp8e4m3 DoubleRow uv matmuls (PTS_UVDR=1: DR streams at 1 elem/cycle
at the pinned 1.2 GHz clock -> slower than fp16, +70% LDWEIGHTS),
draining part of |v| via DVE bitwise-and abs (PTS_BITX=N: per-op DVE
overhead ~150ns eats the ACT savings), GPSIMD minsub (PTS_MINSUB=g:
Q7 tensor_scalar is ~7.5us per [128,512] tile, 25x slower than DVE),
and a PE warmup burst (PTS_WARMUP_MM: the HAM clock-gate never opens
in this environment -- PE stays at 1.2 GHz regardless).
"""

import math
import os
import sys

import numpy as np

for _p in ("/opt/trn_rl_repo",):
    if _p not in sys.path and os.path.isdir(_p):
        sys.path.insert(0, _p)

# Geometry (matches the reference module's fixed canvas).
H, W = 256, 384
N_CORES = 8
ROWS_PER_CORE = H // N_CORES            # 32
TILE_ROWS, TILE_COLS = 4, 128           # spatial tile = 512 px
N_BANDS = ROWS_PER_CORE // TILE_ROWS    # 8 row-bands per core
N_BLOCKS = W // TILE_COLS               # 3 col-blocks per row
TILES_PER_CORE = N_BANDS * N_BLOCKS     # 24
TILE_PX = TILE_ROWS * TILE_COLS         # 512
CAP = 128                               # points per point-tile
WIDTH_TO_HEIGHT = 384.0 / 256.0

# Tunables (env overrides for experiments).
WARMUP_MM = int(os.environ.get("PTS_WARMUP_MM", "0"))
MINSUB = os.environ.get("PTS_MINSUB", "v")   # g=gpsimd, v=vector, a=scalar-relu
UVDR = bool(int(os.environ.get("PTS_UVDR", "0")))  # fp8 DoubleRow uv matmuls
BITX = int(os.environ.get("PTS_BITX", "0"))  # v-tail cols drained via DVE bit-and
SIGEARLY = bool(int(os.environ.get("PTS_SIGEARLY", "0")))  # dummy sigmoid up front
UVPAR = bool(int(os.environ.get("PTS_UVPAR", "1")))  # u/v MMs on parallel row-groups
PAIRABS = bool(int(os.environ.get("PTS_PAIRABS", "0")))  # fuse ACT/DVE over chunk pairs

# Set BASS_KERNEL_TRACE=1 to capture an NTFF profile; results land here.
last_run_info = {}


def _hi_lo(x):
    """Split float64 array into fp16 hi + fp16 lo with tiny residual."""
    hi = x.astype(np.float16)
    lo = (x - hi.astype(np.float64)).astype(np.float16)
    return hi, lo


# fp8 DoubleRow term table: (kind, i, j) -> W row = coeff-split i (kind),
# G row = basis-split j of {by=gy+2, bx=gx+2, ones}.  i+j<=3 keeps the
# dropped-term error ~1e-3 in u (tolerance is ~5e-3).
DR_TERMS = ([("y", i, j) for i in range(4) for j in range(4) if i + j <= 4]
            + [("x", i, j) for i in range(4) for j in range(4) if i + j <= 4]
            + [("c", i, 0) for i in range(4)])
DR_K = len(DR_TERMS)          # 24 rows
DR_KI = DR_K // 2             # 12 interleaved row-pairs


def _q8(x):
    import ml_dtypes
    return np.asarray(x, np.float64).astype(ml_dtypes.float8_e4m3).astype(np.float64)


def _cascade8(x, n=4):
    """e4m3 residual cascade: x ~= sum of n terms, each a power-of-2
    multiple of an e4m3 value (scaled into the normal range before
    quantizing -- e4m3's subnormal floor at 2^-9 otherwise stalls the
    cascade around 1e-3)."""
    outs = []
    r = np.asarray(x, np.float64)
    for _ in range(n):
        m = np.abs(r).max()
        s = 1.0 if m <= 1e-30 else 2.0 ** int(np.floor(np.log2(128.0 / m)))
        t = _q8(r * s) / s
        outs.append(t)
        r = r - t
    return outs


def _prepare(locations, matrix_offsets, matrix_scale_exponents, colors):
    """Host-side prep: per-point combos, culling, per-core packed arrays."""
    loc = np.asarray(locations, np.float64).reshape(-1, 2)      # (N, 2) y,x
    mo = np.asarray(matrix_offsets, np.float64)                  # (N, 2, 2)
    mse = np.asarray(matrix_scale_exponents, np.float64).reshape(-1)
    cols = np.asarray(colors, np.float64).reshape(-1, 3)         # (N, 3)
    n = loc.shape[0]

    scale = (math.sqrt(n) / 2.0) / np.exp(mse)
    mats = mo + np.eye(2)[None, :, :] * scale[:, None, None]     # (N, 2, 2)
    # b_j = loc_y*M[0,j] + loc_x*M[1,j]
    b = loc[:, 0, None] * mats[:, 0, :] + loc[:, 1, None] * mats[:, 1, :]

    wy_u = mats[:, 0, 0] + mats[:, 0, 1]
    wx_u = mats[:, 1, 0] + mats[:, 1, 1]
    c_u = -(b[:, 0] + b[:, 1])
    wy_v = mats[:, 0, 0] - mats[:, 0, 1]
    wx_v = mats[:, 1, 0] - mats[:, 1, 1]
    c_v = -(b[:, 0] - b[:, 1])

    # sigma_min of each 2x2 (exact closed form).
    a_, b_, c_, d_ = mats[:, 0, 0], mats[:, 0, 1], mats[:, 1, 0], mats[:, 1, 1]
    S = a_ * a_ + b_ * b_ + c_ * c_ + d_ * d_
    D = a_ * d_ - b_ * c_
    smin = np.sqrt(np.maximum((S - np.sqrt(np.maximum(S * S - 4 * D * D, 0.0))) / 2.0, 0.0))
    reach = 2.0 / np.maximum(smin, 1e-12) + 1e-5   # small safety margin

    ys = np.linspace(-1.0, 1.0, H).astype(np.float32).astype(np.float64)
    xs = np.linspace(-WIDTH_TO_HEIGHT, WIDTH_TO_HEIGHT, W).astype(np.float32).astype(np.float64)
    gyh, gyl = _hi_lo(ys)
    gxh, gxl = _hi_lo(xs)

    wyu_h, wyu_l = _hi_lo(wy_u)
    wxu_h, wxu_l = _hi_lo(wx_u)
    cu_h, cu_l = _hi_lo(c_u)
    wyv_h, wyv_l = _hi_lo(wy_v)
    wxv_h, wxv_l = _hi_lo(wx_v)
    cv_h, cv_l = _hi_lo(c_v)

    # Per (core, tile): list of candidate point indices.
    tile_pts = [[None] * TILES_PER_CORE for _ in range(N_CORES)]
    max_cnt = 0
    for core in range(N_CORES):
        for t in range(TILES_PER_CORE):
            r, blk = divmod(t, N_BLOCKS)
            r0 = core * ROWS_PER_CORE + r * TILE_ROWS
            ylo, yhi = ys[r0], ys[r0 + TILE_ROWS - 1]
            xlo, xhi = xs[blk * TILE_COLS], xs[blk * TILE_COLS + TILE_COLS - 1]
            dy = np.maximum(np.maximum(ylo - loc[:, 0], loc[:, 0] - yhi), 0.0)
            dx = np.maximum(np.maximum(xlo - loc[:, 1], loc[:, 1] - xhi), 0.0)
            idx = np.nonzero(np.hypot(dy, dx) <= reach)[0]
            tile_pts[core][t] = idx
            max_cnt = max(max_cnt, len(idx))

    # Same program runs on every core -> chunk count per tile slot must be
    # uniform across cores.
    nchunks = [
        max(max(1, -(-len(tile_pts[c][t]) // CAP)) for c in range(N_CORES))
        for t in range(TILES_PER_CORE)
    ]
    chunk_of_tile = []   # flat chunk list: (tile_idx, chunk_idx)
    for t in range(TILES_PER_CORE):
        for k in range(nchunks[t]):
            chunk_of_tile.append((t, k))
    n_chunk = len(chunk_of_tile)

    # fp8 DoubleRow packing (see DR_TERMS).
    if UVDR:
        import ml_dtypes
        f8 = ml_dtypes.float8_e4m3
        by_c = _cascade8(ys + 2.0)          # 4x [H], in [1,3]
        bx_c = _cascade8(xs + 2.0)          # 4x [W]
        cc_u = c_u - 2.0 * wy_u - 2.0 * wx_u
        cc_v = c_v - 2.0 * wy_v - 2.0 * wx_v
        wcas = {
            ("u", "y"): _cascade8(wy_u), ("u", "x"): _cascade8(wx_u),
            ("u", "c"): _cascade8(cc_u),
            ("v", "y"): _cascade8(wy_v), ("v", "x"): _cascade8(wx_v),
            ("v", "c"): _cascade8(cc_v),
        }
        gval = {"y": by_c, "x": bx_c}
        kexp = []
        for kind, i, j in DR_TERMS:
            amax = max(np.abs(wcas[("u", kind)][i]).max(),
                       np.abs(wcas[("v", kind)][i]).max(), 1e-30)
            bmax = 1.0 if kind == "c" else max(np.abs(gval[kind][j]).max(), 1e-30)
            k = int(round(0.5 * (np.log2(bmax) - np.log2(amax))))
            while amax * 2.0 ** k > 224:
                k -= 1
            while bmax * 2.0 ** -k > 224:
                k += 1
            kexp.append(k)

        w8 = np.zeros((N_CORES, DR_KI, n_chunk, 2, 2 * CAP), f8)
        g8 = np.zeros((N_CORES, DR_KI, n_chunk, 2, TILE_PX), f8)
        ct_np = np.zeros((N_CORES, CAP, n_chunk * 3), np.float16)
        csign = 0.5 if MINSUB == "a" else -0.5
        for core in range(N_CORES):
            for ci, (t, k) in enumerate(chunk_of_tile):
                r, blk = divmod(t, N_BLOCKS)
                r0 = core * ROWS_PER_CORE + r * TILE_ROWS
                idx = tile_pts[core][t][k * CAP:(k + 1) * CAP]
                m = len(idx)
                if m:
                    ct_np[core, :m, 3 * ci:3 * ci + 3] = (
                        csign * cols[idx]).astype(np.float16)
                for r_idx, (kind, i, j) in enumerate(DR_TERMS):
                    ki, ko = divmod(r_idx, 2)
                    sc = 2.0 ** kexp[r_idx]
                    if m:
                        w8[core, ki, ci, ko, 0:m] = _q8(
                            wcas[("u", kind)][i][idx] * sc)
                        w8[core, ki, ci, ko, CAP:CAP + m] = _q8(
                            wcas[("v", kind)][i][idx] * sc)
                    if kind == "y":
                        gv = np.repeat(by_c[j][r0:r0 + TILE_ROWS], TILE_COLS)
                    elif kind == "x":
                        gv = np.tile(bx_c[j][blk * TILE_COLS:(blk + 1) * TILE_COLS],
                                     TILE_ROWS)
                    else:
                        gv = np.ones(TILE_PX)
                    g8[core, ki, ci, ko, :] = _q8(gv / sc)
        return w8, g8, ct_np, chunk_of_tile, n_chunk

    # Packed per-core arrays.
    w_np = np.zeros((N_CORES, 8, n_chunk * 2 * CAP), np.float16)
    g_np = np.zeros((N_CORES, 8, n_chunk * TILE_PX), np.float16)
    ct_np = np.zeros((N_CORES, CAP, n_chunk * 3), np.float16)

    # color scale/sign per the minsub variant: m'' = min(d,2)-2 uses -c/2;
    # the ACT-relu variant computes m' = relu(2-d) = -(m'') and uses +c/2.
    csign = 0.5 if MINSUB == "a" else -0.5

    for core in range(N_CORES):
        for ci, (t, k) in enumerate(chunk_of_tile):
            r, blk = divmod(t, N_BLOCKS)
            r0 = core * ROWS_PER_CORE + r * TILE_ROWS
            idx = tile_pts[core][t][k * CAP:(k + 1) * CAP]
            m = len(idx)
            # Weights [8, CAP] for u at cols [2ci*CAP, ...), v next.
            o = 2 * ci * CAP
            if m:
                w_np[core, 0, o:o + m] = wyu_h[idx]
                w_np[core, 1, o:o + m] = wyu_h[idx]
                w_np[core, 2, o:o + m] = wyu_l[idx]
                w_np[core, 3, o:o + m] = wxu_h[idx]
                w_np[core, 4, o:o + m] = wxu_h[idx]
                w_np[core, 5, o:o + m] = wxu_l[idx]
                w_np[core, 6, o:o + m] = cu_h[idx]
                w_np[core, 7, o:o + m] = cu_l[idx]
                o2 = o + CAP
                w_np[core, 0, o2:o2 + m] = wyv_h[idx]
                w_np[core, 1, o2:o2 + m] = wyv_h[idx]
                w_np[core, 2, o2:o2 + m] = wyv_l[idx]
                w_np[core, 3, o2:o2 + m] = wxv_h[idx]
                w_np[core, 4, o2:o2 + m] = wxv_h[idx]
                w_np[core, 5, o2:o2 + m] = wxv_l[idx]
                w_np[core, 6, o2:o2 + m] = cv_h[idx]
                w_np[core, 7, o2:o2 + m] = cv_l[idx]
                ct_np[core, :m, 3 * ci:3 * ci + 3] = (csign * cols[idx]).astype(np.float16)
            # G rows [8, TILE_PX]: px = rr*TILE_COLS + col (row-major in tile)
            go = ci * TILE_PX
            ty_h = np.repeat(gyh[r0:r0 + TILE_ROWS].astype(np.float16), TILE_COLS)
            ty_l = np.repeat(gyl[r0:r0 + TILE_ROWS].astype(np.float16), TILE_COLS)
            tx_h = np.tile(gxh[blk * TILE_COLS:(blk + 1) * TILE_COLS].astype(np.float16), TILE_ROWS)
            tx_l = np.tile(gxl[blk * TILE_COLS:(blk + 1) * TILE_COLS].astype(np.float16), TILE_ROWS)
            g_np[core, 0, go:go + TILE_PX] = ty_h
            g_np[core, 1, go:go + TILE_PX] = ty_l
            g_np[core, 2, go:go + TILE_PX] = ty_h
            g_np[core, 3, go:go + TILE_PX] = tx_h
            g_np[core, 4, go:go + TILE_PX] = tx_l
            g_np[core, 5, go:go + TILE_PX] = tx_h
            g_np[core, 6, go:go + TILE_PX] = 1.0
            g_np[core, 7, go:go + TILE_PX] = 1.0

    return w_np, g_np, ct_np, chunk_of_tile, n_chunk


def emulate_core_math(w_np_c, g_np_c, ct_np_c, chunk_of_tile):
    """Numpy emulation of the per-core device math (for testing)."""
    canvas = np.zeros((128, ROWS_PER_CORE * N_BLOCKS * 3), np.float64)
    for i, (t, k) in enumerate(chunk_of_tile):
        r, blk = divmod(t, N_BLOCKS)
        if UVDR:
            Wq = w_np_c[:, i].astype(np.float32).reshape(DR_K, 2 * CAP)
            Gq = g_np_c[:, i].astype(np.float32).reshape(DR_K, TILE_PX)
            u = Wq[:, 0:CAP].T @ Gq
            v = Wq[:, CAP:2 * CAP].T @ Gq
        else:
            wo, go = 2 * i * CAP, i * TILE_PX
            Wu = w_np_c[:, wo:wo + CAP].astype(np.float32)
            Wv = w_np_c[:, wo + CAP:wo + 2 * CAP].astype(np.float32)
            G = g_np_c[:, go:go + TILE_PX].astype(np.float32)
            u = Wu.T @ G
            v = Wv.T @ G
        au = np.abs(u).astype(np.float16).astype(np.float32)
        av = np.abs(v).astype(np.float16).astype(np.float32)
        if BITX:
            av[:, TILE_PX - BITX:] = np.abs(v[:, TILE_PX - BITX:])
        d = np.maximum(au, av).astype(np.float16).astype(np.float32)
        if MINSUB == "a":
            m = np.maximum(2.0 - d, 0.0).astype(np.float16).astype(np.float32)
        else:
            m = (np.minimum(d, 2.0) - 2.0).astype(np.float16).astype(np.float32)
        ct = ct_np_c[:, 3 * i:3 * i + 3].astype(np.float32)
        for rr in range(TILE_ROWS):
            lr = r * TILE_ROWS + rr
            off = 3 * (lr * N_BLOCKS + blk)
            blkpx = m[:, rr * TILE_COLS:(rr + 1) * TILE_COLS]
            canvas[:, off:off + 3] += blkpx.T @ ct
    return 1.0 / (1.0 + np.exp(-4.0 * canvas))


def _build_nc(n_chunk, chunk_of_tile):
    """Build the Bass/Tile program (shared by all cores)."""
    from contextlib import ExitStack

    import concourse.bacc as bacc
    import concourse.tile as tile
    from concourse import mybir

    f16 = mybir.dt.float16
    f32 = mybir.dt.float32
    u32 = mybir.dt.uint32
    f8e4 = mybir.dt.float8e4
    nc = bacc.Bacc("TRN2", target_bir_lowering=False, debug=False,
                   num_devices=N_CORES)

    if UVDR:
        w_d = nc.dram_tensor("w", [DR_KI, n_chunk, 2, 2 * CAP], f8e4,
                             kind="ExternalInput")
        g_d = nc.dram_tensor("g", [DR_KI, n_chunk, 2, TILE_PX], f8e4,
                             kind="ExternalInput")
    else:
        w_d = nc.dram_tensor("w", [8, n_chunk * 2 * CAP], f16, kind="ExternalInput")
        g_d = nc.dram_tensor("g", [8, n_chunk * TILE_PX], f16, kind="ExternalInput")
    ct_d = nc.dram_tensor("ct", [CAP, n_chunk * 3], f16, kind="ExternalInput")
    y_d = nc.dram_tensor("y", [128, ROWS_PER_CORE * N_BLOCKS * 3], f32, kind="ExternalOutput")

    with ExitStack() as ctx:
        tc = ctx.enter_context(tile.TileContext(nc))
        const = ctx.enter_context(tc.tile_pool(name="const", bufs=1))
        uvpool = ctx.enter_context(tc.tile_pool(
            name="uv", bufs=(1 if PAIRABS else 3), space="PSUM"))
        cvpool = ctx.enter_context(tc.tile_pool(name="cv", bufs=1, space="PSUM"))
        wupool = ctx.enter_context(tc.tile_pool(name="wu", bufs=1, space="PSUM"))
        dpool = ctx.enter_context(tc.tile_pool(name="d", bufs=3))
        mpool = ctx.enter_context(tc.tile_pool(name="m", bufs=3))
        opool = ctx.enter_context(tc.tile_pool(name="o", bufs=1))

        if UVDR:
            W_sb = const.tile([DR_KI, n_chunk, 2, 2 * CAP], f8e4)
            G_sb = const.tile([DR_KI, n_chunk, 2, TILE_PX], f8e4)
        elif UVPAR:
            # v's operands live at partitions 32..39 so the u and v matmuls
            # land on different PE row-groups and run concurrently.
            W_sb = const.tile([40, n_chunk * 2 * CAP], f16)
            G_sb = const.tile([40, n_chunk * TILE_PX], f16)
        else:
            W_sb = const.tile([8, n_chunk * 2 * CAP], f16)
            G_sb = const.tile([8, n_chunk * TILE_PX], f16)
        CT_sb = const.tile([CAP, n_chunk * 3], f16)
        if BITX:
            mask_sb = const.tile([128, 1], u32)
            nc.vector.memset(mask_sb[:], 0x7FFFFFFF)
        # Split the input DMAs by chunk ranges and interleave W/G pieces so
        # the first chunks' operands land ASAP (the serialized full-tensor
        # DMAs otherwise gate the first matmul by ~7us).  With UVPAR the
        # replica transfers double the issue count on the serialized Sync
        # queue, so use fewer/larger pieces to avoid mid-stream stalls.
        if UVPAR:
            # Small first piece for a fast start.  ALL primary pieces are
            # issued before ANY v-replica (narrow 8-partition DMAs only get
            # 8/128 of the DMA port bandwidth, so replicas are slow); the
            # first UVPAR_FROM chunks run u,v serially on row-group 0 so
            # the replicas are not needed until they have surely landed.
            bounds = [0] + [b for b in (3, 9, 16) if b < n_chunk] + [n_chunk]
        else:
            per = -(-n_chunk // 4)
            bounds = list(range(0, n_chunk, per)) + [n_chunk]
        for p in range(len(bounds) - 1):
            c0, c1 = bounds[p], bounds[p + 1]
            if c0 >= c1:
                continue
            if UVDR:
                nc.sync.dma_start(W_sb[:, c0:c1], w_d[:, c0:c1])
                nc.sync.dma_start(G_sb[:, c0:c1], g_d[:, c0:c1])
            else:
                wo0, wo1 = 2 * c0 * CAP, 2 * c1 * CAP
                go0, go1 = c0 * TILE_PX, c1 * TILE_PX
                nc.sync.dma_start(W_sb[0:8, wo0:wo1], w_d[:, wo0:wo1])
                nc.sync.dma_start(G_sb[0:8, go0:go1], g_d[:, go0:go1])
            if p == 0:
                nc.sync.dma_start(CT_sb[:], ct_d[:])
        if UVPAR and not UVDR:
            nc.sync.dma_start(W_sb[32:40, :], w_d[:])
            nc.sync.dma_start(G_sb[32:40, :], g_d[:])

        # PE warmup: dense back-to-back matmuls on a zeroed tile while the
        # input DMAs are in flight.  ~9 * 427ns cold spans the ~3.4us HAM
        # window so the real matmuls run at 2.4 GHz.
        if WARMUP_MM > 0:
            wz = const.tile([128, 512], f16)
            nc.vector.memset(wz[:], 0.0)
            wps = wupool.tile([128, 512], f32)
            for _ in range(WARMUP_MM):
                nc.tensor.matmul(wps[:], wz[:, 0:128], wz[:],
                                 start=True, stop=True)

        canvas = cvpool.tile([128, ROWS_PER_CORE * N_BLOCKS * 3], f32)

        # chunk index ranges per tile for start/stop flags
        first_chunk = {}
        last_chunk = {}
        for ci, (t, k) in enumerate(chunk_of_tile):
            first_chunk.setdefault(t, ci)
            last_chunk[t] = ci

        if PAIRABS:
            # One manually-cycled 3-slot PSUM region (6 banks).  Chunks at
            # slots 0,1 share a single fused Abs over both slots (amortizes
            # the ~240-cycle ACT per-op overhead); slot 2 is processed solo.
            uvbig = uvpool.tile([128, 3, 2 * TILE_PX], f32, tag="uv")
            vlo = 32 if UVPAR else 0

            def _emit_mms(ci):
                sl = ci % 3
                wo = 2 * ci * CAP
                go = ci * TILE_PX
                nc.tensor.matmul(uvbig[:, sl:sl + 1, 0:TILE_PX],
                                 W_sb[0:8, wo:wo + CAP],
                                 G_sb[0:8, go:go + TILE_PX],
                                 start=True, stop=True)
                nc.tensor.matmul(uvbig[:, sl:sl + 1, TILE_PX:2 * TILE_PX],
                                 W_sb[vlo:vlo + 8, wo + CAP:wo + 2 * CAP],
                                 G_sb[vlo:vlo + 8, go:go + TILE_PX],
                                 start=True, stop=True)

            def _emit_tail(grp, aa):
                for q, ci in enumerate(grp):
                    t, k = chunk_of_tile[ci]
                    r, blk = divmod(t, N_BLOCKS)
                    d_sb = dpool.tile([128, TILE_PX], f16, tag="d")
                    nc.vector.tensor_tensor(d_sb[:],
                                            aa[:, q:q + 1, 0:TILE_PX],
                                            aa[:, q:q + 1, TILE_PX:2 * TILE_PX],
                                            op=mybir.AluOpType.max)
                    m_sb = mpool.tile([128, TILE_PX], f16, tag="m")
                    nc.vector.tensor_scalar(
                        m_sb[:], d_sb[:], 2.0, 2.0,
                        op0=mybir.AluOpType.min, op1=mybir.AluOpType.subtract)
                    for rr in range(TILE_ROWS):
                        lr = r * TILE_ROWS + rr
                        off = 3 * (lr * N_BLOCKS + blk)
                        nc.tensor.matmul(canvas[:, off:off + 3],
                                         m_sb[:, rr * TILE_COLS:(rr + 1) * TILE_COLS],
                                         CT_sb[:, 3 * ci:3 * ci + 3],
                                         start=(ci == first_chunk[t]),
                                         stop=(ci == last_chunk[t]))

            ci = 0
            while ci < n_chunk:
                grp = [ci, ci + 1] if (ci % 3 == 0 and ci + 1 < n_chunk) else [ci]
                for c in grp:
                    _emit_mms(c)
                sl0 = grp[0] % 3
                aa = dpool.tile([128, len(grp), 2 * TILE_PX], f16, tag="aa")
                nc.scalar.activation(aa[:], uvbig[:, sl0:sl0 + len(grp), :],
                                     mybir.ActivationFunctionType.Abs)
                _emit_tail(grp, aa)
                ci += len(grp)

        for ci, (t, k) in enumerate([] if PAIRABS else chunk_of_tile):
            r, blk = divmod(t, N_BLOCKS)
            puv = uvpool.tile([128, 2 * TILE_PX], f32, tag="uv")
            if UVDR:
                nc.tensor.matmul(puv[:, 0:TILE_PX], W_sb[:, ci, :, 0:CAP],
                                 G_sb[:, ci, :, :], start=True, stop=True,
                                 perf_mode=mybir.MatmulPerfMode.DoubleRow)
                nc.tensor.matmul(puv[:, TILE_PX:2 * TILE_PX],
                                 W_sb[:, ci, :, CAP:2 * CAP],
                                 G_sb[:, ci, :, :], start=True, stop=True,
                                 perf_mode=mybir.MatmulPerfMode.DoubleRow)
            else:
                wo = 2 * ci * CAP
                go = ci * TILE_PX
                nc.tensor.matmul(puv[:, 0:TILE_PX], W_sb[0:8, wo:wo + CAP],
                                 G_sb[0:8, go:go + TILE_PX], start=True, stop=True)
                vlo = 32 if (UVPAR and ci >= 18) else 0
                nc.tensor.matmul(puv[:, TILE_PX:2 * TILE_PX],
                                 W_sb[vlo:vlo + 8, wo + CAP:wo + 2 * CAP],
                                 G_sb[vlo:vlo + 8, go:go + TILE_PX],
                                 start=True, stop=True)
            # HW allows only ONE PSUM operand per DVE/ACT op and walrus
            # codegen has no float abs ALU op, so ACT's Abs is the main
            # |.| fold: one Abs over u + the head of v, then DVE max.
            # BITX tail cols of v are drained on DVE via bitwise-and abs
            # (fp32 sign-bit clear) to rebalance ACT vs DVE.
            x = BITX
            aa_sb = dpool.tile([128, 2 * TILE_PX - x], f16, tag="aa")
            nc.scalar.activation(aa_sb[:], puv[:, 0:2 * TILE_PX - x],
                                 mybir.ActivationFunctionType.Abs)
            d_sb = dpool.tile([128, TILE_PX], f16, tag="d")
            if x:
                vb_sb = dpool.tile([128, x], f32, tag="vb")
                nc.vector.tensor_scalar(
                    vb_sb[:].bitcast(u32),
                    puv[:, 2 * TILE_PX - x:2 * TILE_PX].bitcast(u32),
                    0x7FFFFFFF, None, op0=mybir.AluOpType.bitwise_and)
                nc.vector.tensor_tensor(d_sb[:, 0:TILE_PX - x],
                                        aa_sb[:, 0:TILE_PX - x],
                                        aa_sb[:, TILE_PX:2 * TILE_PX - x],
                                        op=mybir.AluOpType.max)
                nc.vector.tensor_tensor(d_sb[:, TILE_PX - x:TILE_PX],
                                        aa_sb[:, TILE_PX - x:TILE_PX],
                                        vb_sb[:], op=mybir.AluOpType.max)
            else:
                nc.vector.tensor_tensor(d_sb[:], aa_sb[:, 0:TILE_PX],
                                        aa_sb[:, TILE_PX:2 * TILE_PX],
                                        op=mybir.AluOpType.max)
            m_sb = mpool.tile([128, TILE_PX], f16, tag="m")
            if MINSUB == "a":
                # m' = relu(2 - d); colors carry +0.5
                nc.scalar.activation(m_sb[:], d_sb[:],
                                     mybir.ActivationFunctionType.Relu,
                                     bias=2.0, scale=-1.0)
            else:
                eng = nc.gpsimd if MINSUB == "g" else nc.vector
                eng.tensor_scalar(
                    m_sb[:], d_sb[:], 2.0, 2.0,
                    op0=mybir.AluOpType.min, op1=mybir.AluOpType.subtract)
            for rr in range(TILE_ROWS):
                lr = r * TILE_ROWS + rr
                off = 3 * (lr * N_BLOCKS + blk)
                nc.tensor.matmul(canvas[:, off:off + 3],
                                 m_sb[:, rr * TILE_COLS:(rr + 1) * TILE_COLS],
                                 CT_sb[:, 3 * ci:3 * ci + 3],
                                 start=(ci == first_chunk[t]),
                                 stop=(ci == last_chunk[t]))

        out_sb = opool.tile([128, ROWS_PER_CORE * N_BLOCKS * 3], f32)
        nc.scalar.activation(out_sb[:], canvas[:],
                             mybir.ActivationFunctionType.Sigmoid, scale=4.0)
        nc.sync.dma_start(y_d[:], out_sb[:])

    nc.compile()
    return nc


def _install_ntff_hook():
    """Provide antenv.axon_hooks if the image lacks it (ctypes shim around
    libaxon_pjrt.so's NRT profile capture). Returns True on success."""
    try:
        from antenv.axon_hooks import get_axon_ntff_profile_hook  # noqa: F401
        return True
    except ImportError:
        pass
    try:
        import contextlib
        import ctypes
        import types

        import antenv

        so_path = "/opt/axon/libaxon_pjrt.so"
        lib = ctypes.CDLL(so_path)
        if not hasattr(lib, "axon_start_nrt_profile"):
            return False
        lib.axon_start_nrt_profile.argtypes = [
            ctypes.POINTER(ctypes.c_int64), ctypes.c_size_t]
        lib.axon_start_nrt_profile.restype = ctypes.c_int64
        lib.axon_stop_nrt_profile.argtypes = [ctypes.c_char_p]
        lib.axon_stop_nrt_profile.restype = ctypes.c_int64

        @contextlib.contextmanager
        def _hook(output_dir, device_ids):
            import jax
            jax.devices()
            if device_ids:
                ids = (ctypes.c_int64 * len(device_ids))(*device_ids)
                rc = lib.axon_start_nrt_profile(ids, len(device_ids))
            else:
                rc = lib.axon_start_nrt_profile(None, 0)
            if rc != 0:
                raise RuntimeError(f"axon_start_nrt_profile rc={rc}")
            try:
                yield
            finally:
                n = lib.axon_stop_nrt_profile(str(output_dir).encode())
                print(f"ntff profile: {n} file(s) -> {output_dir}", file=sys.stderr)

        mod = types.ModuleType("antenv.axon_hooks")
        mod._hook = _hook
        mod.get_axon_ntff_profile_hook = lambda: _hook
        mod.set_axon_ntff_profile_hook = lambda h: None
        sys.modules["antenv.axon_hooks"] = mod
        antenv.axon_hooks = mod
        return True
    except Exception as e:  # pragma: no cover
        print("ntff hook install failed:", e, file=sys.stderr)
        return False


def kernel(locations, matrix_offsets, matrix_scale_exponents, colors,
           canvas_height_px, canvas_width_px):
    assert int(canvas_height_px) == H and int(canvas_width_px) == W

    w_np, g_np, ct_np, chunk_of_tile, n_chunk = _prepare(
        locations, matrix_offsets, matrix_scale_exponents, colors)

    nc = _build_nc(n_chunk, chunk_of_tile)

    from concourse.bass_utils import run_bass_kernel_spmd

    in_maps = [
        {"w": w_np[c], "g": g_np[c], "ct": ct_np[c]} for c in range(N_CORES)
    ]
    trace = bool(int(os.environ.get("BASS_KERNEL_TRACE", "0")))
    if trace:
        trace = _install_ntff_hook()
    try:
        res = run_bass_kernel_spmd(nc, in_maps, core_ids=list(range(N_CORES)),
                                   trace=trace)
    except Exception:
        if not trace:
            raise
        res = run_bass_kernel_spmd(nc, in_maps, core_ids=list(range(N_CORES)),
                                   trace=False)
    last_run_info.clear()
    last_run_info.update(
        exec_time_ns=res.exec_time_ns,
        mean_exec_time_ns=res.mean_exec_time_ns,
        profile_json=res.profile_json,
    )

    out = np.empty((3, H, W), np.float32)
    for c in range(N_CORES):
        y = res.results[c]["y"]                       # [128, 32*3*3]
        arr = y.reshape(128, ROWS_PER_CORE, N_BLOCKS, 3)  # p, lr, blk, ch
        out[:, c * ROWS_PER_CORE:(c + 1) * ROWS_PER_CORE, :] = (
            arr.transpose(3, 1, 2, 0).reshape(3, ROWS_PER_CORE, W))
    return out


# revision 44
# speedup vs baseline: 1.0102x; 1.0102x over previous
"""Trainium2 Bass kernel for the nn_Points problem.

Renders N=1024 anisotropic "diamond" points onto a 3x256x384 canvas:
    t = (pixel - loc) @ M_n          (2-vector per pixel per point)
    mapped = relu(1 - (|t0|+|t1|)/2)
    canvas = sigmoid(4 * sum_n mapped * color_n)

Strategy (8 NeuronCores, full inputs in / full output out):
  * Spatial-shard the canvas: core c renders rows [32c, 32c+32).
  * Within a core: 24 spatial tiles of 4 rows x 128 cols (512 px).
  * Host-side exact culling: point n can touch a tile only if
    sigma_min(M_n) * dist2(loc_n, tile_rect) <= 2  (else |t|_1 >= 2
    everywhere in the tile and mapped is exactly 0).  Measured <= ~82
    points per tile, so one 128-slot point tile per spatial tile.
  * u = t0+t1, v = t0-t1 are affine in (gy, gx) -> computed as K=8
    fp16 matmuls (hi/lo split of coords/consts for fp32-grade accuracy):
        out[pt, px] = W[k, pt].T @ G[k, px]
    The u and v matmuls run CONCURRENTLY on different PE row-groups
    (K=8 uses 8 of 128 array rows; v's operands are replicated at
    partitions 32..39 so its matmul lands on row-group 1 and starts
    ~5ns after u's -- measured ~2x on the uv stream).
  * |u|,|v|: one ACT Abs over both PSUM banks (hardware allows only ONE
    PSUM operand per DVE/ACT instruction and walrus has no float-abs DVE
    ALU op, so ACT's Abs is the only cheap |.| fold); then DVE
    tensor_tensor max and a fused tensor_scalar m'' = min(d,2)-2
    (= -2*relu(1-d/2)); the -0.5 sign/scale is folded into the colors.
  * canvas: matmul with mapped (fp16, SBUF) as the stationary operand,
    colors [128pts, 3] as moving operand; accumulates [128px, 3] blocks
    into one persistent PSUM bank laid out [128, 32rows*3blk*3ch].
  * Input DMAs are split into per-chunk-range pieces (W/G interleaved)
    so the first chunks' operands land ~6us earlier than a monolithic
    transfer would allow; compute starts while later pieces stream in.
  * One sigmoid(4x) ACT over the whole core's canvas + one DMA out.

Measured on 8 axon trn2 cores: 45.3us (baseline 48.8us).  Steady state
is ~1.16us/chunk with ACT (Abs) ~96% and PE ~96% busy simultaneously;
~7us NEFF/engine-init preamble, ~1.5us first-DMA latency and a ~4us
multi-core drain/barrier tail are framework-fixed.  Experiments that
did NOT help on this silicon (kept behind env flags, all default-off):
fp8e4m3 DoubleRow uv matmuls (PTS_UVDR=1: DR streams at 1 elem/cycle
at the pinned 1.2 GHz clock -> slower than fp16, +70% LDWEIGHTS),
draining part of |v| via DVE bitwise-and abs (PTS_BITX=N: per-op DVE
overhead ~150ns eats the ACT savings), GPSIMD minsub (PTS_MINSUB=g:
Q7 tensor_scalar is ~7.5us per [128,512] tile, 25x slower than DVE),
and a PE warmup burst (PTS_WARMUP_MM: the HAM clock-gate never opens
in this environment -- PE stays at 1.2 GHz regardless).
"""

import math
import os
import sys

import numpy as np

for _p in ("/opt/trn_rl_repo",):
    if _p not in sys.path and os.path.isdir(_p):
        sys.path.insert(0, _p)

# Geometry (matches the reference module's fixed canvas).
H, W = 256, 384
N_CORES = 8
ROWS_PER_CORE = H // N_CORES            # 32
TILE_ROWS, TILE_COLS = 4, 128           # spatial tile = 512 px
N_BANDS = ROWS_PER_CORE // TILE_ROWS    # 8 row-bands per core
N_BLOCKS = W // TILE_COLS               # 3 col-blocks per row
TILES_PER_CORE = N_BANDS * N_BLOCKS     # 24
TILE_PX = TILE_ROWS * TILE_COLS         # 512
CAP = 128                               # points per point-tile
WIDTH_TO_HEIGHT = 384.0 / 256.0

# Tunables (env overrides for experiments).
WARMUP_MM = int(os.environ.get("PTS_WARMUP_MM", "0"))
MINSUB = os.environ.get("PTS_MINSUB", "v")   # g=gpsimd, v=vector, a=scalar-relu
UVDR = bool(int(os.environ.get("PTS_UVDR", "0")))  # fp8 DoubleRow uv matmuls
BITX = int(os.environ.get("PTS_BITX", "0"))  # v-tail cols drained via DVE bit-and
SIGEARLY = bool(int(os.environ.get("PTS_SIGEARLY", "0")))  # dummy sigmoid up front
UVPAR = bool(int(os.environ.get("PTS_UVPAR", "1")))  # u/v MMs on parallel row-groups
PAIRABS = bool(int(os.environ.get("PTS_PAIRABS", "0")))  # fuse ACT/DVE over chunk pairs
UVPAR_FROM = int(os.environ.get("PTS_UVPAR_FROM", "12"))  # first chunk with parallel v

# Set BASS_KERNEL_TRACE=1 to capture an NTFF profile; results land here.
last_run_info = {}


def _hi_lo(x):
    """Split float64 array into fp16 hi + fp16 lo with tiny residual."""
    hi = x.astype(np.float16)
    lo = (x - hi.astype(np.float64)).astype(np.float16)
    return hi, lo


# fp8 DoubleRow term table: (kind, i, j) -> W row = coeff-split i (kind),
# G row = basis-split j of {by=gy+2, bx=gx+2, ones}.  i+j<=3 keeps the
# dropped-term error ~1e-3 in u (tolerance is ~5e-3).
DR_TERMS = ([("y", i, j) for i in range(4) for j in range(4) if i + j <= 4]
            + [("x", i, j) for i in range(4) for j in range(4) if i + j <= 4]
            + [("c", i, 0) for i in range(4)])
DR_K = len(DR_TERMS)          # 24 rows
DR_KI = DR_K // 2             # 12 interleaved row-pairs


def _q8(x):
    import ml_dtypes
    return np.asarray(x, np.float64).astype(ml_dtypes.float8_e4m3).astype(np.float64)


def _cascade8(x, n=4):
    """e4m3 residual cascade: x ~= sum of n terms, each a power-of-2
    multiple of an e4m3 value (scaled into the normal range before
    quantizing -- e4m3's subnormal floor at 2^-9 otherwise stalls the
    cascade around 1e-3)."""
    outs = []
    r = np.asarray(x, np.float64)
    for _ in range(n):
        m = np.abs(r).max()
        s = 1.0 if m <= 1e-30 else 2.0 ** int(np.floor(np.log2(128.0 / m)))
        t = _q8(r * s) / s
        outs.append(t)
        r = r - t
    return outs


def _prepare(locations, matrix_offsets, matrix_scale_exponents, colors):
    """Host-side prep: per-point combos, culling, per-core packed arrays."""
    loc = np.asarray(locations, np.float64).reshape(-1, 2)      # (N, 2) y,x
    mo = np.asarray(matrix_offsets, np.float64)                  # (N, 2, 2)
    mse = np.asarray(matrix_scale_exponents, np.float64).reshape(-1)
    cols = np.asarray(colors, np.float64).reshape(-1, 3)         # (N, 3)
    n = loc.shape[0]

    scale = (math.sqrt(n) / 2.0) / np.exp(mse)
    mats = mo + np.eye(2)[None, :, :] * scale[:, None, None]     # (N, 2, 2)
    # b_j = loc_y*M[0,j] + loc_x*M[1,j]
    b = loc[:, 0, None] * mats[:, 0, :] + loc[:, 1, None] * mats[:, 1, :]

    wy_u = mats[:, 0, 0] + mats[:, 0, 1]
    wx_u = mats[:, 1, 0] + mats[:, 1, 1]
    c_u = -(b[:, 0] + b[:, 1])
    wy_v = mats[:, 0, 0] - mats[:, 0, 1]
    wx_v = mats[:, 1, 0] - mats[:, 1, 1]
    c_v = -(b[:, 0] - b[:, 1])

    # sigma_min of each 2x2 (exact closed form).
    a_, b_, c_, d_ = mats[:, 0, 0], mats[:, 0, 1], mats[:, 1, 0], mats[:, 1, 1]
    S = a_ * a_ + b_ * b_ + c_ * c_ + d_ * d_
    D = a_ * d_ - b_ * c_
    smin = np.sqrt(np.maximum((S - np.sqrt(np.maximum(S * S - 4 * D * D, 0.0))) / 2.0, 0.0))
    reach = 2.0 / np.maximum(smin, 1e-12) + 1e-5   # small safety margin

    ys = np.linspace(-1.0, 1.0, H).astype(np.float32).astype(np.float64)
    xs = np.linspace(-WIDTH_TO_HEIGHT, WIDTH_TO_HEIGHT, W).astype(np.float32).astype(np.float64)
    gyh, gyl = _hi_lo(ys)
    gxh, gxl = _hi_lo(xs)

    wyu_h, wyu_l = _hi_lo(wy_u)
    wxu_h, wxu_l = _hi_lo(wx_u)
    cu_h, cu_l = _hi_lo(c_u)
    wyv_h, wyv_l = _hi_lo(wy_v)
    wxv_h, wxv_l = _hi_lo(wx_v)
    cv_h, cv_l = _hi_lo(c_v)

    # Per (core, tile): list of candidate point indices.
    tile_pts = [[None] * TILES_PER_CORE for _ in range(N_CORES)]
    max_cnt = 0
    for core in range(N_CORES):
        for t in range(TILES_PER_CORE):
            r, blk = divmod(t, N_BLOCKS)
            r0 = core * ROWS_PER_CORE + r * TILE_ROWS
            ylo, yhi = ys[r0], ys[r0 + TILE_ROWS - 1]
            xlo, xhi = xs[blk * TILE_COLS], xs[blk * TILE_COLS + TILE_COLS - 1]
            dy = np.maximum(np.maximum(ylo - loc[:, 0], loc[:, 0] - yhi), 0.0)
            dx = np.maximum(np.maximum(xlo - loc[:, 1], loc[:, 1] - xhi), 0.0)
            idx = np.nonzero(np.hypot(dy, dx) <= reach)[0]
            tile_pts[core][t] = idx
            max_cnt = max(max_cnt, len(idx))

    # Same program runs on every core -> chunk count per tile slot must be
    # uniform across cores.
    nchunks = [
        max(max(1, -(-len(tile_pts[c][t]) // CAP)) for c in range(N_CORES))
        for t in range(TILES_PER_CORE)
    ]
    chunk_of_tile = []   # flat chunk list: (tile_idx, chunk_idx)
    for t in range(TILES_PER_CORE):
        for k in range(nchunks[t]):
            chunk_of_tile.append((t, k))
    n_chunk = len(chunk_of_tile)

    # fp8 DoubleRow packing (see DR_TERMS).
    if UVDR:
        import ml_dtypes
        f8 = ml_dtypes.float8_e4m3
        by_c = _cascade8(ys + 2.0)          # 4x [H], in [1,3]
        bx_c = _cascade8(xs + 2.0)          # 4x [W]
        cc_u = c_u - 2.0 * wy_u - 2.0 * wx_u
        cc_v = c_v - 2.0 * wy_v - 2.0 * wx_v
        wcas = {
            ("u", "y"): _cascade8(wy_u), ("u", "x"): _cascade8(wx_u),
            ("u", "c"): _cascade8(cc_u),
            ("v", "y"): _cascade8(wy_v), ("v", "x"): _cascade8(wx_v),
            ("v", "c"): _cascade8(cc_v),
        }
        gval = {"y": by_c, "x": bx_c}
        kexp = []
        for kind, i, j in DR_TERMS:
            amax = max(np.abs(wcas[("u", kind)][i]).max(),
                       np.abs(wcas[("v", kind)][i]).max(), 1e-30)
            bmax = 1.0 if kind == "c" else max(np.abs(gval[kind][j]).max(), 1e-30)
            k = int(round(0.5 * (np.log2(bmax) - np.log2(amax))))
            while amax * 2.0 ** k > 224:
                k -= 1
            while bmax * 2.0 ** -k > 224:
                k += 1
            kexp.append(k)

        w8 = np.zeros((N_CORES, DR_KI, n_chunk, 2, 2 * CAP), f8)
        g8 = np.zeros((N_CORES, DR_KI, n_chunk, 2, TILE_PX), f8)
        ct_np = np.zeros((N_CORES, CAP, n_chunk * 3), np.float16)
        csign = 0.5 if MINSUB == "a" else -0.5
        for core in range(N_CORES):
            for ci, (t, k) in enumerate(chunk_of_tile):
                r, blk = divmod(t, N_BLOCKS)
                r0 = core * ROWS_PER_CORE + r * TILE_ROWS
                idx = tile_pts[core][t][k * CAP:(k + 1) * CAP]
                m = len(idx)
                if m:
                    ct_np[core, :m, 3 * ci:3 * ci + 3] = (
                        csign * cols[idx]).astype(np.float16)
                for r_idx, (kind, i, j) in enumerate(DR_TERMS):
                    ki, ko = divmod(r_idx, 2)
                    sc = 2.0 ** kexp[r_idx]
                    if m:
                        w8[core, ki, ci, ko, 0:m] = _q8(
                            wcas[("u", kind)][i][idx] * sc)
                        w8[core, ki, ci, ko, CAP:CAP + m] = _q8(
                            wcas[("v", kind)][i][idx] * sc)
                    if kind == "y":
                        gv = np.repeat(by_c[j][r0:r0 + TILE_ROWS], TILE_COLS)
                    elif kind == "x":
                        gv = np.tile(bx_c[j][blk * TILE_COLS:(blk + 1) * TILE_COLS],
                                     TILE_ROWS)
                    else:
                        gv = np.ones(TILE_PX)
                    g8[core, ki, ci, ko, :] = _q8(gv / sc)
        return w8, g8, ct_np, chunk_of_tile, n_chunk

    # Packed per-core arrays.
    w_np = np.zeros((N_CORES, 8, n_chunk * 2 * CAP), np.float16)
    g_np = np.zeros((N_CORES, 8, n_chunk * TILE_PX), np.float16)
    ct_np = np.zeros((N_CORES, CAP, n_chunk * 3), np.float16)

    # color scale/sign per the minsub variant: m'' = min(d,2)-2 uses -c/2;
    # the ACT-relu variant computes m' = relu(2-d) = -(m'') and uses +c/2.
    csign = 0.5 if MINSUB == "a" else -0.5

    for core in range(N_CORES):
        for ci, (t, k) in enumerate(chunk_of_tile):
            r, blk = divmod(t, N_BLOCKS)
            r0 = core * ROWS_PER_CORE + r * TILE_ROWS
            idx = tile_pts[core][t][k * CAP:(k + 1) * CAP]
            m = len(idx)
            # Weights [8, CAP] for u at cols [2ci*CAP, ...), v next.
            o = 2 * ci * CAP
            if m:
                w_np[core, 0, o:o + m] = wyu_h[idx]
                w_np[core, 1, o:o + m] = wyu_h[idx]
                w_np[core, 2, o:o + m] = wyu_l[idx]
                w_np[core, 3, o:o + m] = wxu_h[idx]
                w_np[core, 4, o:o + m] = wxu_h[idx]
                w_np[core, 5, o:o + m] = wxu_l[idx]
                w_np[core, 6, o:o + m] = cu_h[idx]
                w_np[core, 7, o:o + m] = cu_l[idx]
                o2 = o + CAP
                w_np[core, 0, o2:o2 + m] = wyv_h[idx]
                w_np[core, 1, o2:o2 + m] = wyv_h[idx]
                w_np[core, 2, o2:o2 + m] = wyv_l[idx]
                w_np[core, 3, o2:o2 + m] = wxv_h[idx]
                w_np[core, 4, o2:o2 + m] = wxv_h[idx]
                w_np[core, 5, o2:o2 + m] = wxv_l[idx]
                w_np[core, 6, o2:o2 + m] = cv_h[idx]
                w_np[core, 7, o2:o2 + m] = cv_l[idx]
                ct_np[core, :m, 3 * ci:3 * ci + 3] = (csign * cols[idx]).astype(np.float16)
            # G rows [8, TILE_PX]: px = rr*TILE_COLS + col (row-major in tile)
            go = ci * TILE_PX
            ty_h = np.repeat(gyh[r0:r0 + TILE_ROWS].astype(np.float16), TILE_COLS)
            ty_l = np.repeat(gyl[r0:r0 + TILE_ROWS].astype(np.float16), TILE_COLS)
            tx_h = np.tile(gxh[blk * TILE_COLS:(blk + 1) * TILE_COLS].astype(np.float16), TILE_ROWS)
            tx_l = np.tile(gxl[blk * TILE_COLS:(blk + 1) * TILE_COLS].astype(np.float16), TILE_ROWS)
            g_np[core, 0, go:go + TILE_PX] = ty_h
            g_np[core, 1, go:go + TILE_PX] = ty_l
            g_np[core, 2, go:go + TILE_PX] = ty_h
            g_np[core, 3, go:go + TILE_PX] = tx_h
            g_np[core, 4, go:go + TILE_PX] = tx_l
            g_np[core, 5, go:go + TILE_PX] = tx_h
            g_np[core, 6, go:go + TILE_PX] = 1.0
            g_np[core, 7, go:go + TILE_PX] = 1.0

    return w_np, g_np, ct_np, chunk_of_tile, n_chunk


def emulate_core_math(w_np_c, g_np_c, ct_np_c, chunk_of_tile):
    """Numpy emulation of the per-core device math (for testing)."""
    canvas = np.zeros((128, ROWS_PER_CORE * N_BLOCKS * 3), np.float64)
    for i, (t, k) in enumerate(chunk_of_tile):
        r, blk = divmod(t, N_BLOCKS)
        if UVDR:
            Wq = w_np_c[:, i].astype(np.float32).reshape(DR_K, 2 * CAP)
            Gq = g_np_c[:, i].astype(np.float32).reshape(DR_K, TILE_PX)
            u = Wq[:, 0:CAP].T @ Gq
            v = Wq[:, CAP:2 * CAP].T @ Gq
        else:
            wo, go = 2 * i * CAP, i * TILE_PX
            Wu = w_np_c[:, wo:wo + CAP].astype(np.float32)
            Wv = w_np_c[:, wo + CAP:wo + 2 * CAP].astype(np.float32)
            G = g_np_c[:, go:go + TILE_PX].astype(np.float32)
            u = Wu.T @ G
            v = Wv.T @ G
        au = np.abs(u).astype(np.float16).astype(np.float32)
        av = np.abs(v).astype(np.float16).astype(np.float32)
        if BITX:
            av[:, TILE_PX - BITX:] = np.abs(v[:, TILE_PX - BITX:])
        d = np.maximum(au, av).astype(np.float16).astype(np.float32)
        if MINSUB == "a":
            m = np.maximum(2.0 - d, 0.0).astype(np.float16).astype(np.float32)
        else:
            m = (np.minimum(d, 2.0) - 2.0).astype(np.float16).astype(np.float32)
        ct = ct_np_c[:, 3 * i:3 * i + 3].astype(np.float32)
        for rr in range(TILE_ROWS):
            lr = r * TILE_ROWS + rr
            off = 3 * (lr * N_BLOCKS + blk)
            blkpx = m[:, rr * TILE_COLS:(rr + 1) * TILE_COLS]
            canvas[:, off:off + 3] += blkpx.T @ ct
    return 1.0 / (1.0 + np.exp(-4.0 * canvas))


def _build_nc(n_chunk, chunk_of_tile):
    """Build the Bass/Tile program (shared by all cores)."""
    from contextlib import ExitStack

    import concourse.bacc as bacc
    import concourse.tile as tile
    from concourse import mybir

    f16 = mybir.dt.float16
    f32 = mybir.dt.float32
    u32 = mybir.dt.uint32
    f8e4 = mybir.dt.float8e4
    nc = bacc.Bacc("TRN2", target_bir_lowering=False, debug=False,
                   num_devices=N_CORES)

    if UVDR:
        w_d = nc.dram_tensor("w", [DR_KI, n_chunk, 2, 2 * CAP], f8e4,
                             kind="ExternalInput")
        g_d = nc.dram_tensor("g", [DR_KI, n_chunk, 2, TILE_PX], f8e4,
                             kind="ExternalInput")
    else:
        w_d = nc.dram_tensor("w", [8, n_chunk * 2 * CAP], f16, kind="ExternalInput")
        g_d = nc.dram_tensor("g", [8, n_chunk * TILE_PX], f16, kind="ExternalInput")
    ct_d = nc.dram_tensor("ct", [CAP, n_chunk * 3], f16, kind="ExternalInput")
    y_d = nc.dram_tensor("y", [128, ROWS_PER_CORE * N_BLOCKS * 3], f32, kind="ExternalOutput")

    with ExitStack() as ctx:
        tc = ctx.enter_context(tile.TileContext(nc))
        const = ctx.enter_context(tc.tile_pool(name="const", bufs=1))
        uvpool = ctx.enter_context(tc.tile_pool(
            name="uv", bufs=(1 if PAIRABS else 3), space="PSUM"))
        cvpool = ctx.enter_context(tc.tile_pool(name="cv", bufs=1, space="PSUM"))
        wupool = ctx.enter_context(tc.tile_pool(name="wu", bufs=1, space="PSUM"))
        dpool = ctx.enter_context(tc.tile_pool(name="d", bufs=3))
        mpool = ctx.enter_context(tc.tile_pool(name="m", bufs=3))
        opool = ctx.enter_context(tc.tile_pool(name="o", bufs=1))

        if UVDR:
            W_sb = const.tile([DR_KI, n_chunk, 2, 2 * CAP], f8e4)
            G_sb = const.tile([DR_KI, n_chunk, 2, TILE_PX], f8e4)
        elif UVPAR:
            # v's operands live at partitions 32..39 so the u and v matmuls
            # land on different PE row-groups and run concurrently.
            W_sb = const.tile([40, n_chunk * 2 * CAP], f16)
            G_sb = const.tile([40, n_chunk * TILE_PX], f16)
        else:
            W_sb = const.tile([8, n_chunk * 2 * CAP], f16)
            G_sb = const.tile([8, n_chunk * TILE_PX], f16)
        CT_sb = const.tile([CAP, n_chunk * 3], f16)
        if BITX:
            mask_sb = const.tile([128, 1], u32)
            nc.vector.memset(mask_sb[:], 0x7FFFFFFF)
        # Split the input DMAs by chunk ranges and interleave W/G pieces so
        # the first chunks' operands land ASAP (the serialized full-tensor
        # DMAs otherwise gate the first matmul by ~7us).  With UVPAR the
        # replica transfers double the issue count on the serialized Sync
        # queue, so use fewer/larger pieces to avoid mid-stream stalls.
        if UVPAR:
            # Small first piece for a fast start.  ALL primary pieces are
            # issued before ANY v-replica (narrow 8-partition DMAs only get
            # 8/128 of the DMA port bandwidth, so replicas are slow); the
            # first UVPAR_FROM chunks run u,v serially on row-group 0 so
            # the replicas are not needed until they have surely landed.
            bounds = [0] + [b for b in (3, 9, 16) if b < n_chunk] + [n_chunk]
        else:
            per = -(-n_chunk // 4)
            bounds = list(range(0, n_chunk, per)) + [n_chunk]
        for p in range(len(bounds) - 1):
            c0, c1 = bounds[p], bounds[p + 1]
            if c0 >= c1:
                continue
            if UVDR:
                nc.sync.dma_start(W_sb[:, c0:c1], w_d[:, c0:c1])
                nc.sync.dma_start(G_sb[:, c0:c1], g_d[:, c0:c1])
            else:
                wo0, wo1 = 2 * c0 * CAP, 2 * c1 * CAP
                go0, go1 = c0 * TILE_PX, c1 * TILE_PX
                nc.sync.dma_start(W_sb[0:8, wo0:wo1], w_d[:, wo0:wo1])
                nc.sync.dma_start(G_sb[0:8, go0:go1], g_d[:, go0:go1])
            if p == 0:
                nc.sync.dma_start(CT_sb[:], ct_d[:])
        if UVPAR and not UVDR:
            # Replicate only the chunks that run v in parallel: the narrow
            # 8-partition replica DMA moves ~15GB/s (8/128 of the ports),
            # so replica bytes directly set how early the switch can be.
            wr0, gr0 = 2 * UVPAR_FROM * CAP, UVPAR_FROM * TILE_PX
            nc.sync.dma_start(W_sb[32:40, wr0:], w_d[:, wr0:])
            nc.sync.dma_start(G_sb[32:40, gr0:], g_d[:, gr0:])

        # PE warmup: dense back-to-back matmuls on a zeroed tile while the
        # input DMAs are in flight.  ~9 * 427ns cold spans the ~3.4us HAM
        # window so the real matmuls run at 2.4 GHz.
        if WARMUP_MM > 0:
            wz = const.tile([128, 512], f16)
            nc.vector.memset(wz[:], 0.0)
            wps = wupool.tile([128, 512], f32)
            for _ in range(WARMUP_MM):
                nc.tensor.matmul(wps[:], wz[:, 0:128], wz[:],
                                 start=True, stop=True)

        canvas = cvpool.tile([128, ROWS_PER_CORE * N_BLOCKS * 3], f32)

        # chunk index ranges per tile for start/stop flags
        first_chunk = {}
        last_chunk = {}
        for ci, (t, k) in enumerate(chunk_of_tile):
            first_chunk.setdefault(t, ci)
            last_chunk[t] = ci

        if PAIRABS:
            # One manually-cycled 3-slot PSUM region (6 banks).  Chunks at
            # slots 0,1 share a single fused Abs over both slots (amortizes
            # the ~240-cycle ACT per-op overhead); slot 2 is processed solo.
            uvbig = uvpool.tile([128, 3, 2 * TILE_PX], f32, tag="uv")
            vlo = 32 if UVPAR else 0

            def _emit_mms(ci):
                sl = ci % 3
                wo = 2 * ci * CAP
                go = ci * TILE_PX
                nc.tensor.matmul(uvbig[:, sl:sl + 1, 0:TILE_PX],
                                 W_sb[0:8, wo:wo + CAP],
                                 G_sb[0:8, go:go + TILE_PX],
                                 start=True, stop=True)
                nc.tensor.matmul(uvbig[:, sl:sl + 1, TILE_PX:2 * TILE_PX],
                                 W_sb[vlo:vlo + 8, wo + CAP:wo + 2 * CAP],
                                 G_sb[vlo:vlo + 8, go:go + TILE_PX],
                                 start=True, stop=True)

            def _emit_tail(grp, aa):
                for q, ci in enumerate(grp):
                    t, k = chunk_of_tile[ci]
                    r, blk = divmod(t, N_BLOCKS)
                    d_sb = dpool.tile([128, TILE_PX], f16, tag="d")
                    nc.vector.tensor_tensor(d_sb[:],
                                            aa[:, q:q + 1, 0:TILE_PX],
                                            aa[:, q:q + 1, TILE_PX:2 * TILE_PX],
                                            op=mybir.AluOpType.max)
                    m_sb = mpool.tile([128, TILE_PX], f16, tag="m")
                    nc.vector.tensor_scalar(
                        m_sb[:], d_sb[:], 2.0, 2.0,
                        op0=mybir.AluOpType.min, op1=mybir.AluOpType.subtract)
                    for rr in range(TILE_ROWS):
                        lr = r * TILE_ROWS + rr
                        off = 3 * (lr * N_BLOCKS + blk)
                        nc.tensor.matmul(canvas[:, off:off + 3],
                                         m_sb[:, rr * TILE_COLS:(rr + 1) * TILE_COLS],
                                         CT_sb[:, 3 * ci:3 * ci + 3],
                                         start=(ci == first_chunk[t]),
                                         stop=(ci == last_chunk[t]))

            ci = 0
            while ci < n_chunk:
                grp = [ci, ci + 1] if (ci % 3 == 0 and ci + 1 < n_chunk) else [ci]
                for c in grp:
                    _emit_mms(c)
                sl0 = grp[0] % 3
                aa = dpool.tile([128, len(grp), 2 * TILE_PX], f16, tag="aa")
                nc.scalar.activation(aa[:], uvbig[:, sl0:sl0 + len(grp), :],
                                     mybir.ActivationFunctionType.Abs)
                _emit_tail(grp, aa)
                ci += len(grp)

        for ci, (t, k) in enumerate([] if PAIRABS else chunk_of_tile):
            r, blk = divmod(t, N_BLOCKS)
            puv = uvpool.tile([128, 2 * TILE_PX], f32, tag="uv")
            if UVDR:
                nc.tensor.matmul(puv[:, 0:TILE_PX], W_sb[:, ci, :, 0:CAP],
                                 G_sb[:, ci, :, :], start=True, stop=True,
                                 perf_mode=mybir.MatmulPerfMode.DoubleRow)
                nc.tensor.matmul(puv[:, TILE_PX:2 * TILE_PX],
                                 W_sb[:, ci, :, CAP:2 * CAP],
                                 G_sb[:, ci, :, :], start=True, stop=True,
                                 perf_mode=mybir.MatmulPerfMode.DoubleRow)
            else:
                wo = 2 * ci * CAP
                go = ci * TILE_PX
                nc.tensor.matmul(puv[:, 0:TILE_PX], W_sb[0:8, wo:wo + CAP],
                                 G_sb[0:8, go:go + TILE_PX], start=True, stop=True)
                vlo = 32 if (UVPAR and ci >= UVPAR_FROM) else 0
                nc.tensor.matmul(puv[:, TILE_PX:2 * TILE_PX],
                                 W_sb[vlo:vlo + 8, wo + CAP:wo + 2 * CAP],
                                 G_sb[vlo:vlo + 8, go:go + TILE_PX],
                                 start=True, stop=True)
            # HW allows only ONE PSUM operand per DVE/ACT op and walrus
            # codegen has no float abs ALU op, so ACT's Abs is the main
            # |.| fold: one Abs over u + the head of v, then DVE max.
            # BITX tail cols of v are drained on DVE via bitwise-and abs
            # (fp32 sign-bit clear) to rebalance ACT vs DVE.
            x = BITX
            aa_sb = dpool.tile([128, 2 * TILE_PX - x], f16, tag="aa")
            nc.scalar.activation(aa_sb[:], puv[:, 0:2 * TILE_PX - x],
                                 mybir.ActivationFunctionType.Abs)
            d_sb = dpool.tile([128, TILE_PX], f16, tag="d")
            if x:
                vb_sb = dpool.tile([128, x], f32, tag="vb")
                nc.vector.tensor_scalar(
                    vb_sb[:].bitcast(u32),
                    puv[:, 2 * TILE_PX - x:2 * TILE_PX].bitcast(u32),
                    0x7FFFFFFF, None, op0=mybir.AluOpType.bitwise_and)
                nc.vector.tensor_tensor(d_sb[:, 0:TILE_PX - x],
                                        aa_sb[:, 0:TILE_PX - x],
                                        aa_sb[:, TILE_PX:2 * TILE_PX - x],
                                        op=mybir.AluOpType.max)
                nc.vector.tensor_tensor(d_sb[:, TILE_PX - x:TILE_PX],
                                        aa_sb[:, TILE_PX - x:TILE_PX],
                                        vb_sb[:], op=mybir.AluOpType.max)
            else:
                nc.vector.tensor_tensor(d_sb[:], aa_sb[:, 0:TILE_PX],
                                        aa_sb[:, TILE_PX:2 * TILE_PX],
                                        op=mybir.AluOpType.max)
            m_sb = mpool.tile([128, TILE_PX], f16, tag="m")
            if MINSUB == "a":
                # m' = relu(2 - d); colors carry +0.5
                nc.scalar.activation(m_sb[:], d_sb[:],
                                     mybir.ActivationFunctionType.Relu,
                                     bias=2.0, scale=-1.0)
            else:
                eng = nc.gpsimd if MINSUB == "g" else nc.vector
                eng.tensor_scalar(
                    m_sb[:], d_sb[:], 2.0, 2.0,
                    op0=mybir.AluOpType.min, op1=mybir.AluOpType.subtract)
            for rr in range(TILE_ROWS):
                lr = r * TILE_ROWS + rr
                off = 3 * (lr * N_BLOCKS + blk)
                nc.tensor.matmul(canvas[:, off:off + 3],
                                 m_sb[:, rr * TILE_COLS:(rr + 1) * TILE_COLS],
                                 CT_sb[:, 3 * ci:3 * ci + 3],
                                 start=(ci == first_chunk[t]),
                                 stop=(ci == last_chunk[t]))

        out_sb = opool.tile([128, ROWS_PER_CORE * N_BLOCKS * 3], f32)
        nc.scalar.activation(out_sb[:], canvas[:],
                             mybir.ActivationFunctionType.Sigmoid, scale=4.0)
        nc.sync.dma_start(y_d[:], out_sb[:])

    nc.compile()
    return nc


def _install_ntff_hook():
    """Provide antenv.axon_hooks if the image lacks it (ctypes shim around
    libaxon_pjrt.so's NRT profile capture). Returns True on success."""
    try:
        from antenv.axon_hooks import get_axon_ntff_profile_hook  # noqa: F401
        return True
    except ImportError:
        pass
    try:
        import contextlib
        import ctypes
        import types

        import antenv

        so_path = "/opt/axon/libaxon_pjrt.so"
        lib = ctypes.CDLL(so_path)
        if not hasattr(lib, "axon_start_nrt_profile"):
            return False
        lib.axon_start_nrt_profile.argtypes = [
            ctypes.POINTER(ctypes.c_int64), ctypes.c_size_t]
        lib.axon_start_nrt_profile.restype = ctypes.c_int64
        lib.axon_stop_nrt_profile.argtypes = [ctypes.c_char_p]
        lib.axon_stop_nrt_profile.restype = ctypes.c_int64

        @contextlib.contextmanager
        def _hook(output_dir, device_ids):
            import jax
            jax.devices()
            if device_ids:
                ids = (ctypes.c_int64 * len(device_ids))(*device_ids)
                rc = lib.axon_start_nrt_profile(ids, len(device_ids))
            else:
                rc = lib.axon_start_nrt_profile(None, 0)
            if rc != 0:
                raise RuntimeError(f"axon_start_nrt_profile rc={rc}")
            try:
                yield
            finally:
                n = lib.axon_stop_nrt_profile(str(output_dir).encode())
                print(f"ntff profile: {n} file(s) -> {output_dir}", file=sys.stderr)

        mod = types.ModuleType("antenv.axon_hooks")
        mod._hook = _hook
        mod.get_axon_ntff_profile_hook = lambda: _hook
        mod.set_axon_ntff_profile_hook = lambda h: None
        sys.modules["antenv.axon_hooks"] = mod
        antenv.axon_hooks = mod
        return True
    except Exception as e:  # pragma: no cover
        print("ntff hook install failed:", e, file=sys.stderr)
        return False


def kernel(locations, matrix_offsets, matrix_scale_exponents, colors,
           canvas_height_px, canvas_width_px):
    assert int(canvas_height_px) == H and int(canvas_width_px) == W

    w_np, g_np, ct_np, chunk_of_tile, n_chunk = _prepare(
        locations, matrix_offsets, matrix_scale_exponents, colors)

    nc = _build_nc(n_chunk, chunk_of_tile)

    from concourse.bass_utils import run_bass_kernel_spmd

    in_maps = [
        {"w": w_np[c], "g": g_np[c], "ct": ct_np[c]} for c in range(N_CORES)
    ]
    trace = bool(int(os.environ.get("BASS_KERNEL_TRACE", "0")))
    if trace:
        trace = _install_ntff_hook()
    try:
        res = run_bass_kernel_spmd(nc, in_maps, core_ids=list(range(N_CORES)),
                                   trace=trace)
    except Exception:
        if not trace:
            raise
        res = run_bass_kernel_spmd(nc, in_maps, core_ids=list(range(N_CORES)),
                                   trace=False)
    last_run_info.clear()
    last_run_info.update(
        exec_time_ns=res.exec_time_ns,
        mean_exec_time_ns=res.mean_exec_time_ns,
        profile_json=res.profile_json,
    )

    out = np.empty((3, H, W), np.float32)
    for c in range(N_CORES):
        y = res.results[c]["y"]                       # [128, 32*3*3]
        arr = y.reshape(128, ROWS_PER_CORE, N_BLOCKS, 3)  # p, lr, blk, ch
        out[:, c * ROWS_PER_CORE:(c + 1) * ROWS_PER_CORE, :] = (
            arr.transpose(3, 1, 2, 0).reshape(3, ROWS_PER_CORE, W))
    return out


# revision 45
# speedup vs baseline: 1.0131x; 1.0029x over previous
"""Trainium2 Bass kernel for the nn_Points problem.

Renders N=1024 anisotropic "diamond" points onto a 3x256x384 canvas:
    t = (pixel - loc) @ M_n          (2-vector per pixel per point)
    mapped = relu(1 - (|t0|+|t1|)/2)
    canvas = sigmoid(4 * sum_n mapped * color_n)

Strategy (8 NeuronCores, full inputs in / full output out):
  * Spatial-shard the canvas: core c renders rows [32c, 32c+32).
  * Within a core: 24 spatial tiles of 4 rows x 128 cols (512 px).
  * Host-side exact culling: point n can touch a tile only if
    sigma_min(M_n) * dist2(loc_n, tile_rect) <= 2  (else |t|_1 >= 2
    everywhere in the tile and mapped is exactly 0).  Measured <= ~82
    points per tile, so one 128-slot point tile per spatial tile.
  * u = t0+t1, v = t0-t1 are affine in (gy, gx) -> computed as K=8
    fp16 matmuls (hi/lo split of coords/consts for fp32-grade accuracy):
        out[pt, px] = W[k, pt].T @ G[k, px]
    The u and v matmuls run CONCURRENTLY on different PE row-groups
    (K=8 uses 8 of 128 array rows; v's operands are replicated at
    partitions 32..39 so its matmul lands on row-group 1 and starts
    ~5ns after u's -- measured ~2x on the uv stream).
  * |u|,|v|: one ACT Abs over both PSUM banks (hardware allows only ONE
    PSUM operand per DVE/ACT instruction and walrus has no float-abs DVE
    ALU op, so ACT's Abs is the only cheap |.| fold); then DVE
    tensor_tensor max and a fused tensor_scalar m'' = min(d,2)-2
    (= -2*relu(1-d/2)); the -0.5 sign/scale is folded into the colors.
  * canvas: matmul with mapped (fp16, SBUF) as the stationary operand,
    colors [128pts, 3] as moving operand; accumulates [128px, 3] blocks
    into one persistent PSUM bank laid out [128, 32rows*3blk*3ch].
  * Input DMAs are split into per-chunk-range pieces (W/G interleaved)
    so the first chunks' operands land ~6us earlier than a monolithic
    transfer would allow; compute starts while later pieces stream in.
  * One sigmoid(4x) ACT over the whole core's canvas + one DMA out.

Measured on 8 axon trn2 cores: 45.3us (baseline 48.8us).  Steady state
is ~1.16us/chunk with ACT (Abs) ~96% and PE ~96% busy simultaneously;
~7us NEFF/engine-init preamble, ~1.5us first-DMA latency and a ~4us
multi-core drain/barrier tail are framework-fixed.  Experiments that
did NOT help on this silicon (kept behind env flags, all default-off):
fp8e4m3 DoubleRow uv matmuls (PTS_UVDR=1: DR streams at 1 elem/cycle
at the pinned 1.2 GHz clock -> slower than fp16, +70% LDWEIGHTS),
draining part of |v| via DVE bitwise-and abs (PTS_BITX=N: per-op DVE
overhead ~150ns eats the ACT savings), GPSIMD minsub (PTS_MINSUB=g:
Q7 tensor_scalar is ~7.5us per [128,512] tile, 25x slower than DVE),
and a PE warmup burst (PTS_WARMUP_MM: the HAM clock-gate never opens
in this environment -- PE stays at 1.2 GHz regardless).
"""

import math
import os
import sys

import numpy as np

for _p in ("/opt/trn_rl_repo",):
    if _p not in sys.path and os.path.isdir(_p):
        sys.path.insert(0, _p)

# Geometry (matches the reference module's fixed canvas).
H, W = 256, 384
N_CORES = 8
ROWS_PER_CORE = H // N_CORES            # 32
TILE_ROWS, TILE_COLS = 4, 128           # spatial tile = 512 px
N_BANDS = ROWS_PER_CORE // TILE_ROWS    # 8 row-bands per core
N_BLOCKS = W // TILE_COLS               # 3 col-blocks per row
TILES_PER_CORE = N_BANDS * N_BLOCKS     # 24
TILE_PX = TILE_ROWS * TILE_COLS         # 512
CAP = 128                               # points per point-tile
WIDTH_TO_HEIGHT = 384.0 / 256.0

# Tunables (env overrides for experiments).
WARMUP_MM = int(os.environ.get("PTS_WARMUP_MM", "0"))
MINSUB = os.environ.get("PTS_MINSUB", "v")   # g=gpsimd, v=vector, a=scalar-relu
UVDR = bool(int(os.environ.get("PTS_UVDR", "0")))  # fp8 DoubleRow uv matmuls
BITX = int(os.environ.get("PTS_BITX", "0"))  # v-tail cols drained via DVE bit-and
SIGEARLY = bool(int(os.environ.get("PTS_SIGEARLY", "0")))  # dummy sigmoid up front
UVPAR = bool(int(os.environ.get("PTS_UVPAR", "1")))  # u/v MMs on parallel row-groups
PAIRABS = bool(int(os.environ.get("PTS_PAIRABS", "0")))  # fuse ACT/DVE over chunk pairs
UVPAR_FROM = int(os.environ.get("PTS_UVPAR_FROM", "14"))  # first chunk with parallel v

# Set BASS_KERNEL_TRACE=1 to capture an NTFF profile; results land here.
last_run_info = {}


def _hi_lo(x):
    """Split float64 array into fp16 hi + fp16 lo with tiny residual."""
    hi = x.astype(np.float16)
    lo = (x - hi.astype(np.float64)).astype(np.float16)
    return hi, lo


# fp8 DoubleRow term table: (kind, i, j) -> W row = coeff-split i (kind),
# G row = basis-split j of {by=gy+2, bx=gx+2, ones}.  i+j<=3 keeps the
# dropped-term error ~1e-3 in u (tolerance is ~5e-3).
DR_TERMS = ([("y", i, j) for i in range(4) for j in range(4) if i + j <= 4]
            + [("x", i, j) for i in range(4) for j in range(4) if i + j <= 4]
            + [("c", i, 0) for i in range(4)])
DR_K = len(DR_TERMS)          # 24 rows
DR_KI = DR_K // 2             # 12 interleaved row-pairs


def _q8(x):
    import ml_dtypes
    return np.asarray(x, np.float64).astype(ml_dtypes.float8_e4m3).astype(np.float64)


def _cascade8(x, n=4):
    """e4m3 residual cascade: x ~= sum of n terms, each a power-of-2
    multiple of an e4m3 value (scaled into the normal range before
    quantizing -- e4m3's subnormal floor at 2^-9 otherwise stalls the
    cascade around 1e-3)."""
    outs = []
    r = np.asarray(x, np.float64)
    for _ in range(n):
        m = np.abs(r).max()
        s = 1.0 if m <= 1e-30 else 2.0 ** int(np.floor(np.log2(128.0 / m)))
        t = _q8(r * s) / s
        outs.append(t)
        r = r - t
    return outs


def _prepare(locations, matrix_offsets, matrix_scale_exponents, colors):
    """Host-side prep: per-point combos, culling, per-core packed arrays."""
    loc = np.asarray(locations, np.float64).reshape(-1, 2)      # (N, 2) y,x
    mo = np.asarray(matrix_offsets, np.float64)                  # (N, 2, 2)
    mse = np.asarray(matrix_scale_exponents, np.float64).reshape(-1)
    cols = np.asarray(colors, np.float64).reshape(-1, 3)         # (N, 3)
    n = loc.shape[0]

    scale = (math.sqrt(n) / 2.0) / np.exp(mse)
    mats = mo + np.eye(2)[None, :, :] * scale[:, None, None]     # (N, 2, 2)
    # b_j = loc_y*M[0,j] + loc_x*M[1,j]
    b = loc[:, 0, None] * mats[:, 0, :] + loc[:, 1, None] * mats[:, 1, :]

    wy_u = mats[:, 0, 0] + mats[:, 0, 1]
    wx_u = mats[:, 1, 0] + mats[:, 1, 1]
    c_u = -(b[:, 0] + b[:, 1])
    wy_v = mats[:, 0, 0] - mats[:, 0, 1]
    wx_v = mats[:, 1, 0] - mats[:, 1, 1]
    c_v = -(b[:, 0] - b[:, 1])

    # sigma_min of each 2x2 (exact closed form).
    a_, b_, c_, d_ = mats[:, 0, 0], mats[:, 0, 1], mats[:, 1, 0], mats[:, 1, 1]
    S = a_ * a_ + b_ * b_ + c_ * c_ + d_ * d_
    D = a_ * d_ - b_ * c_
    smin = np.sqrt(np.maximum((S - np.sqrt(np.maximum(S * S - 4 * D * D, 0.0))) / 2.0, 0.0))
    reach = 2.0 / np.maximum(smin, 1e-12) + 1e-5   # small safety margin

    ys = np.linspace(-1.0, 1.0, H).astype(np.float32).astype(np.float64)
    xs = np.linspace(-WIDTH_TO_HEIGHT, WIDTH_TO_HEIGHT, W).astype(np.float32).astype(np.float64)
    gyh, gyl = _hi_lo(ys)
    gxh, gxl = _hi_lo(xs)

    wyu_h, wyu_l = _hi_lo(wy_u)
    wxu_h, wxu_l = _hi_lo(wx_u)
    cu_h, cu_l = _hi_lo(c_u)
    wyv_h, wyv_l = _hi_lo(wy_v)
    wxv_h, wxv_l = _hi_lo(wx_v)
    cv_h, cv_l = _hi_lo(c_v)

    # Per (core, tile): list of candidate point indices.
    tile_pts = [[None] * TILES_PER_CORE for _ in range(N_CORES)]
    max_cnt = 0
    for core in range(N_CORES):
        for t in range(TILES_PER_CORE):
            r, blk = divmod(t, N_BLOCKS)
            r0 = core * ROWS_PER_CORE + r * TILE_ROWS
            ylo, yhi = ys[r0], ys[r0 + TILE_ROWS - 1]
            xlo, xhi = xs[blk * TILE_COLS], xs[blk * TILE_COLS + TILE_COLS - 1]
            dy = np.maximum(np.maximum(ylo - loc[:, 0], loc[:, 0] - yhi), 0.0)
            dx = np.maximum(np.maximum(xlo - loc[:, 1], loc[:, 1] - xhi), 0.0)
            idx = np.nonzero(np.hypot(dy, dx) <= reach)[0]
            tile_pts[core][t] = idx
            max_cnt = max(max_cnt, len(idx))

    # Same program runs on every core -> chunk count per tile slot must be
    # uniform across cores.
    nchunks = [
        max(max(1, -(-len(tile_pts[c][t]) // CAP)) for c in range(N_CORES))
        for t in range(TILES_PER_CORE)
    ]
    chunk_of_tile = []   # flat chunk list: (tile_idx, chunk_idx)
    for t in range(TILES_PER_CORE):
        for k in range(nchunks[t]):
            chunk_of_tile.append((t, k))
    n_chunk = len(chunk_of_tile)

    # fp8 DoubleRow packing (see DR_TERMS).
    if UVDR:
        import ml_dtypes
        f8 = ml_dtypes.float8_e4m3
        by_c = _cascade8(ys + 2.0)          # 4x [H], in [1,3]
        bx_c = _cascade8(xs + 2.0)          # 4x [W]
        cc_u = c_u - 2.0 * wy_u - 2.0 * wx_u
        cc_v = c_v - 2.0 * wy_v - 2.0 * wx_v
        wcas = {
            ("u", "y"): _cascade8(wy_u), ("u", "x"): _cascade8(wx_u),
            ("u", "c"): _cascade8(cc_u),
            ("v", "y"): _cascade8(wy_v), ("v", "x"): _cascade8(wx_v),
            ("v", "c"): _cascade8(cc_v),
        }
        gval = {"y": by_c, "x": bx_c}
        kexp = []
        for kind, i, j in DR_TERMS:
            amax = max(np.abs(wcas[("u", kind)][i]).max(),
                       np.abs(wcas[("v", kind)][i]).max(), 1e-30)
            bmax = 1.0 if kind == "c" else max(np.abs(gval[kind][j]).max(), 1e-30)
            k = int(round(0.5 * (np.log2(bmax) - np.log2(amax))))
            while amax * 2.0 ** k > 224:
                k -= 1
            while bmax * 2.0 ** -k > 224:
                k += 1
            kexp.append(k)

        w8 = np.zeros((N_CORES, DR_KI, n_chunk, 2, 2 * CAP), f8)
        g8 = np.zeros((N_CORES, DR_KI, n_chunk, 2, TILE_PX), f8)
        ct_np = np.zeros((N_CORES, CAP, n_chunk * 3), np.float16)
        csign = 0.5 if MINSUB == "a" else -0.5
        for core in range(N_CORES):
            for ci, (t, k) in enumerate(chunk_of_tile):
                r, blk = divmod(t, N_BLOCKS)
                r0 = core * ROWS_PER_CORE + r * TILE_ROWS
                idx = tile_pts[core][t][k * CAP:(k + 1) * CAP]
                m = len(idx)
                if m:
                    ct_np[core, :m, 3 * ci:3 * ci + 3] = (
                        csign * cols[idx]).astype(np.float16)
                for r_idx, (kind, i, j) in enumerate(DR_TERMS):
                    ki, ko = divmod(r_idx, 2)
                    sc = 2.0 ** kexp[r_idx]
                    if m:
                        w8[core, ki, ci, ko, 0:m] = _q8(
                            wcas[("u", kind)][i][idx] * sc)
                        w8[core, ki, ci, ko, CAP:CAP + m] = _q8(
                            wcas[("v", kind)][i][idx] * sc)
                    if kind == "y":
                        gv = np.repeat(by_c[j][r0:r0 + TILE_ROWS], TILE_COLS)
                    elif kind == "x":
                        gv = np.tile(bx_c[j][blk * TILE_COLS:(blk + 1) * TILE_COLS],
                                     TILE_ROWS)
                    else:
                        gv = np.ones(TILE_PX)
                    g8[core, ki, ci, ko, :] = _q8(gv / sc)
        return w8, g8, ct_np, chunk_of_tile, n_chunk

    # Packed per-core arrays.
    w_np = np.zeros((N_CORES, 8, n_chunk * 2 * CAP), np.float16)
    g_np = np.zeros((N_CORES, 8, n_chunk * TILE_PX), np.float16)
    ct_np = np.zeros((N_CORES, CAP, n_chunk * 3), np.float16)

    # color scale/sign per the minsub variant: m'' = min(d,2)-2 uses -c/2;
    # the ACT-relu variant computes m' = relu(2-d) = -(m'') and uses +c/2.
    csign = 0.5 if MINSUB == "a" else -0.5

    for core in range(N_CORES):
        for ci, (t, k) in enumerate(chunk_of_tile):
            r, blk = divmod(t, N_BLOCKS)
            r0 = core * ROWS_PER_CORE + r * TILE_ROWS
            idx = tile_pts[core][t][k * CAP:(k + 1) * CAP]
            m = len(idx)
            # Weights [8, CAP] for u at cols [2ci*CAP, ...), v next.
            o = 2 * ci * CAP
            if m:
                w_np[core, 0, o:o + m] = wyu_h[idx]
                w_np[core, 1, o:o + m] = wyu_h[idx]
                w_np[core, 2, o:o + m] = wyu_l[idx]
                w_np[core, 3, o:o + m] = wxu_h[idx]
                w_np[core, 4, o:o + m] = wxu_h[idx]
                w_np[core, 5, o:o + m] = wxu_l[idx]
                w_np[core, 6, o:o + m] = cu_h[idx]
                w_np[core, 7, o:o + m] = cu_l[idx]
                o2 = o + CAP
                w_np[core, 0, o2:o2 + m] = wyv_h[idx]
                w_np[core, 1, o2:o2 + m] = wyv_h[idx]
                w_np[core, 2, o2:o2 + m] = wyv_l[idx]
                w_np[core, 3, o2:o2 + m] = wxv_h[idx]
                w_np[core, 4, o2:o2 + m] = wxv_h[idx]
                w_np[core, 5, o2:o2 + m] = wxv_l[idx]
                w_np[core, 6, o2:o2 + m] = cv_h[idx]
                w_np[core, 7, o2:o2 + m] = cv_l[idx]
                ct_np[core, :m, 3 * ci:3 * ci + 3] = (csign * cols[idx]).astype(np.float16)
            # G rows [8, TILE_PX]: px = rr*TILE_COLS + col (row-major in tile)
            go = ci * TILE_PX
            ty_h = np.repeat(gyh[r0:r0 + TILE_ROWS].astype(np.float16), TILE_COLS)
            ty_l = np.repeat(gyl[r0:r0 + TILE_ROWS].astype(np.float16), TILE_COLS)
            tx_h = np.tile(gxh[blk * TILE_COLS:(blk + 1) * TILE_COLS].astype(np.float16), TILE_ROWS)
            tx_l = np.tile(gxl[blk * TILE_COLS:(blk + 1) * TILE_COLS].astype(np.float16), TILE_ROWS)
            g_np[core, 0, go:go + TILE_PX] = ty_h
            g_np[core, 1, go:go + TILE_PX] = ty_l
            g_np[core, 2, go:go + TILE_PX] = ty_h
            g_np[core, 3, go:go + TILE_PX] = tx_h
            g_np[core, 4, go:go + TILE_PX] = tx_l
            g_np[core, 5, go:go + TILE_PX] = tx_h
            g_np[core, 6, go:go + TILE_PX] = 1.0
            g_np[core, 7, go:go + TILE_PX] = 1.0

    return w_np, g_np, ct_np, chunk_of_tile, n_chunk


def emulate_core_math(w_np_c, g_np_c, ct_np_c, chunk_of_tile):
    """Numpy emulation of the per-core device math (for testing)."""
    canvas = np.zeros((128, ROWS_PER_CORE * N_BLOCKS * 3), np.float64)
    for i, (t, k) in enumerate(chunk_of_tile):
        r, blk = divmod(t, N_BLOCKS)
        if UVDR:
            Wq = w_np_c[:, i].astype(np.float32).reshape(DR_K, 2 * CAP)
            Gq = g_np_c[:, i].astype(np.float32).reshape(DR_K, TILE_PX)
            u = Wq[:, 0:CAP].T @ Gq
            v = Wq[:, CAP:2 * CAP].T @ Gq
        else:
            wo, go = 2 * i * CAP, i * TILE_PX
            Wu = w_np_c[:, wo:wo + CAP].astype(np.float32)
            Wv = w_np_c[:, wo + CAP:wo + 2 * CAP].astype(np.float32)
            G = g_np_c[:, go:go + TILE_PX].astype(np.float32)
            u = Wu.T @ G
            v = Wv.T @ G
        au = np.abs(u).astype(np.float16).astype(np.float32)
        av = np.abs(v).astype(np.float16).astype(np.float32)
        if BITX:
            av[:, TILE_PX - BITX:] = np.abs(v[:, TILE_PX - BITX:])
        d = np.maximum(au, av).astype(np.float16).astype(np.float32)
        if MINSUB == "a":
            m = np.maximum(2.0 - d, 0.0).astype(np.float16).astype(np.float32)
        else:
            m = (np.minimum(d, 2.0) - 2.0).astype(np.float16).astype(np.float32)
        ct = ct_np_c[:, 3 * i:3 * i + 3].astype(np.float32)
        for rr in range(TILE_ROWS):
            lr = r * TILE_ROWS + rr
            off = 3 * (lr * N_BLOCKS + blk)
            blkpx = m[:, rr * TILE_COLS:(rr + 1) * TILE_COLS]
            canvas[:, off:off + 3] += blkpx.T @ ct
    return 1.0 / (1.0 + np.exp(-4.0 * canvas))


def _build_nc(n_chunk, chunk_of_tile):
    """Build the Bass/Tile program (shared by all cores)."""
    from contextlib import ExitStack

    import concourse.bacc as bacc
    import concourse.tile as tile
    from concourse import mybir

    f16 = mybir.dt.float16
    f32 = mybir.dt.float32
    u32 = mybir.dt.uint32
    f8e4 = mybir.dt.float8e4
    nc = bacc.Bacc("TRN2", target_bir_lowering=False, debug=False,
                   num_devices=N_CORES)

    if UVDR:
        w_d = nc.dram_tensor("w", [DR_KI, n_chunk, 2, 2 * CAP], f8e4,
                             kind="ExternalInput")
        g_d = nc.dram_tensor("g", [DR_KI, n_chunk, 2, TILE_PX], f8e4,
                             kind="ExternalInput")
    else:
        w_d = nc.dram_tensor("w", [8, n_chunk * 2 * CAP], f16, kind="ExternalInput")
        g_d = nc.dram_tensor("g", [8, n_chunk * TILE_PX], f16, kind="ExternalInput")
    ct_d = nc.dram_tensor("ct", [CAP, n_chunk * 3], f16, kind="ExternalInput")
    y_d = nc.dram_tensor("y", [128, ROWS_PER_CORE * N_BLOCKS * 3], f32, kind="ExternalOutput")

    with ExitStack() as ctx:
        tc = ctx.enter_context(tile.TileContext(nc))
        const = ctx.enter_context(tc.tile_pool(name="const", bufs=1))
        uvpool = ctx.enter_context(tc.tile_pool(
            name="uv", bufs=(1 if PAIRABS else 3), space="PSUM"))
        cvpool = ctx.enter_context(tc.tile_pool(name="cv", bufs=1, space="PSUM"))
        wupool = ctx.enter_context(tc.tile_pool(name="wu", bufs=1, space="PSUM"))
        dpool = ctx.enter_context(tc.tile_pool(name="d", bufs=3))
        mpool = ctx.enter_context(tc.tile_pool(name="m", bufs=3))
        opool = ctx.enter_context(tc.tile_pool(name="o", bufs=1))

        if UVDR:
            W_sb = const.tile([DR_KI, n_chunk, 2, 2 * CAP], f8e4)
            G_sb = const.tile([DR_KI, n_chunk, 2, TILE_PX], f8e4)
        elif UVPAR:
            # v's operands live at partitions 32..39 so the u and v matmuls
            # land on different PE row-groups and run concurrently.
            W_sb = const.tile([40, n_chunk * 2 * CAP], f16)
            G_sb = const.tile([40, n_chunk * TILE_PX], f16)
        else:
            W_sb = const.tile([8, n_chunk * 2 * CAP], f16)
            G_sb = const.tile([8, n_chunk * TILE_PX], f16)
        CT_sb = const.tile([CAP, n_chunk * 3], f16)
        if BITX:
            mask_sb = const.tile([128, 1], u32)
            nc.vector.memset(mask_sb[:], 0x7FFFFFFF)
        # Split the input DMAs by chunk ranges and interleave W/G pieces so
        # the first chunks' operands land ASAP (the serialized full-tensor
        # DMAs otherwise gate the first matmul by ~7us).  With UVPAR the
        # replica transfers double the issue count on the serialized Sync
        # queue, so use fewer/larger pieces to avoid mid-stream stalls.
        if UVPAR:
            # Small first piece for a fast start.  ALL primary pieces are
            # issued before ANY v-replica (narrow 8-partition DMAs only get
            # 8/128 of the DMA port bandwidth, so replicas are slow); the
            # first UVPAR_FROM chunks run u,v serially on row-group 0 so
            # the replicas are not needed until they have surely landed.
            bounds = [0] + [b for b in (3, 9, 16) if b < n_chunk] + [n_chunk]
        else:
            per = -(-n_chunk // 4)
            bounds = list(range(0, n_chunk, per)) + [n_chunk]
        for p in range(len(bounds) - 1):
            c0, c1 = bounds[p], bounds[p + 1]
            if c0 >= c1:
                continue
            if UVDR:
                nc.sync.dma_start(W_sb[:, c0:c1], w_d[:, c0:c1])
                nc.sync.dma_start(G_sb[:, c0:c1], g_d[:, c0:c1])
            else:
                wo0, wo1 = 2 * c0 * CAP, 2 * c1 * CAP
                go0, go1 = c0 * TILE_PX, c1 * TILE_PX
                nc.sync.dma_start(W_sb[0:8, wo0:wo1], w_d[:, wo0:wo1])
                nc.sync.dma_start(G_sb[0:8, go0:go1], g_d[:, go0:go1])
            if p == 0:
                nc.sync.dma_start(CT_sb[:], ct_d[:])
        if UVPAR and not UVDR:
            # Replicate only the chunks that run v in parallel: the narrow
            # 8-partition replica DMA moves ~15GB/s (8/128 of the ports),
            # so replica bytes directly set how early the switch can be.
            wr0, gr0 = 2 * UVPAR_FROM * CAP, UVPAR_FROM * TILE_PX
            nc.sync.dma_start(W_sb[32:40, wr0:], w_d[:, wr0:])
            nc.sync.dma_start(G_sb[32:40, gr0:], g_d[:, gr0:])

        # PE warmup: dense back-to-back matmuls on a zeroed tile while the
        # input DMAs are in flight.  ~9 * 427ns cold spans the ~3.4us HAM
        # window so the real matmuls run at 2.4 GHz.
        if WARMUP_MM > 0:
            wz = const.tile([128, 512], f16)
            nc.vector.memset(wz[:], 0.0)
            wps = wupool.tile([128, 512], f32)
            for _ in range(WARMUP_MM):
                nc.tensor.matmul(wps[:], wz[:, 0:128], wz[:],
                                 start=True, stop=True)

        canvas = cvpool.tile([128, ROWS_PER_CORE * N_BLOCKS * 3], f32)

        # chunk index ranges per tile for start/stop flags
        first_chunk = {}
        last_chunk = {}
        for ci, (t, k) in enumerate(chunk_of_tile):
            first_chunk.setdefault(t, ci)
            last_chunk[t] = ci

        if PAIRABS:
            # One manually-cycled 3-slot PSUM region (6 banks).  Chunks at
            # slots 0,1 share a single fused Abs over both slots (amortizes
            # the ~240-cycle ACT per-op overhead); slot 2 is processed solo.
            uvbig = uvpool.tile([128, 3, 2 * TILE_PX], f32, tag="uv")
            vlo = 32 if UVPAR else 0

            def _emit_mms(ci):
                sl = ci % 3
                wo = 2 * ci * CAP
                go = ci * TILE_PX
                nc.tensor.matmul(uvbig[:, sl:sl + 1, 0:TILE_PX],
                                 W_sb[0:8, wo:wo + CAP],
                                 G_sb[0:8, go:go + TILE_PX],
                                 start=True, stop=True)
                nc.tensor.matmul(uvbig[:, sl:sl + 1, TILE_PX:2 * TILE_PX],
                                 W_sb[vlo:vlo + 8, wo + CAP:wo + 2 * CAP],
                                 G_sb[vlo:vlo + 8, go:go + TILE_PX],
                                 start=True, stop=True)

            def _emit_tail(grp, aa):
                for q, ci in enumerate(grp):
                    t, k = chunk_of_tile[ci]
                    r, blk = divmod(t, N_BLOCKS)
                    d_sb = dpool.tile([128, TILE_PX], f16, tag="d")
                    nc.vector.tensor_tensor(d_sb[:],
                                            aa[:, q:q + 1, 0:TILE_PX],
                                            aa[:, q:q + 1, TILE_PX:2 * TILE_PX],
                                            op=mybir.AluOpType.max)
                    m_sb = mpool.tile([128, TILE_PX], f16, tag="m")
                    nc.vector.tensor_scalar(
                        m_sb[:], d_sb[:], 2.0, 2.0,
                        op0=mybir.AluOpType.min, op1=mybir.AluOpType.subtract)
                    for rr in range(TILE_ROWS):
                        lr = r * TILE_ROWS + rr
                        off = 3 * (lr * N_BLOCKS + blk)
                        nc.tensor.matmul(canvas[:, off:off + 3],
                                         m_sb[:, rr * TILE_COLS:(rr + 1) * TILE_COLS],
                                         CT_sb[:, 3 * ci:3 * ci + 3],
                                         start=(ci == first_chunk[t]),
                                         stop=(ci == last_chunk[t]))

            ci = 0
            while ci < n_chunk:
                grp = [ci, ci + 1] if (ci % 3 == 0 and ci + 1 < n_chunk) else [ci]
                for c in grp:
                    _emit_mms(c)
                sl0 = grp[0] % 3
                aa = dpool.tile([128, len(grp), 2 * TILE_PX], f16, tag="aa")
                nc.scalar.activation(aa[:], uvbig[:, sl0:sl0 + len(grp), :],
                                     mybir.ActivationFunctionType.Abs)
                _emit_tail(grp, aa)
                ci += len(grp)

        for ci, (t, k) in enumerate([] if PAIRABS else chunk_of_tile):
            r, blk = divmod(t, N_BLOCKS)
            puv = uvpool.tile([128, 2 * TILE_PX], f32, tag="uv")
            if UVDR:
                nc.tensor.matmul(puv[:, 0:TILE_PX], W_sb[:, ci, :, 0:CAP],
                                 G_sb[:, ci, :, :], start=True, stop=True,
                                 perf_mode=mybir.MatmulPerfMode.DoubleRow)
                nc.tensor.matmul(puv[:, TILE_PX:2 * TILE_PX],
                                 W_sb[:, ci, :, CAP:2 * CAP],
                                 G_sb[:, ci, :, :], start=True, stop=True,
                                 perf_mode=mybir.MatmulPerfMode.DoubleRow)
            else:
                wo = 2 * ci * CAP
                go = ci * TILE_PX
                nc.tensor.matmul(puv[:, 0:TILE_PX], W_sb[0:8, wo:wo + CAP],
                                 G_sb[0:8, go:go + TILE_PX], start=True, stop=True)
                vlo = 32 if (UVPAR and ci >= UVPAR_FROM) else 0
                nc.tensor.matmul(puv[:, TILE_PX:2 * TILE_PX],
                                 W_sb[vlo:vlo + 8, wo + CAP:wo + 2 * CAP],
                                 G_sb[vlo:vlo + 8, go:go + TILE_PX],
                                 start=True, stop=True)
            # HW allows only ONE PSUM operand per DVE/ACT op and walrus
            # codegen has no float abs ALU op, so ACT's Abs is the main
            # |.| fold: one Abs over u + the head of v, then DVE max.
            # BITX tail cols of v are drained on DVE via bitwise-and abs
            # (fp32 sign-bit clear) to rebalance ACT vs DVE.
            x = BITX
            aa_sb = dpool.tile([128, 2 * TILE_PX - x], f16, tag="aa")
            nc.scalar.activation(aa_sb[:], puv[:, 0:2 * TILE_PX - x],
                                 mybir.ActivationFunctionType.Abs)
            d_sb = dpool.tile([128, TILE_PX], f16, tag="d")
            if x:
                vb_sb = dpool.tile([128, x], f32, tag="vb")
                nc.vector.tensor_scalar(
                    vb_sb[:].bitcast(u32),
                    puv[:, 2 * TILE_PX - x:2 * TILE_PX].bitcast(u32),
                    0x7FFFFFFF, None, op0=mybir.AluOpType.bitwise_and)
                nc.vector.tensor_tensor(d_sb[:, 0:TILE_PX - x],
                                        aa_sb[:, 0:TILE_PX - x],
                                        aa_sb[:, TILE_PX:2 * TILE_PX - x],
                                        op=mybir.AluOpType.max)
                nc.vector.tensor_tensor(d_sb[:, TILE_PX - x:TILE_PX],
                                        aa_sb[:, TILE_PX - x:TILE_PX],
                                        vb_sb[:], op=mybir.AluOpType.max)
            else:
                nc.vector.tensor_tensor(d_sb[:], aa_sb[:, 0:TILE_PX],
                                        aa_sb[:, TILE_PX:2 * TILE_PX],
                                        op=mybir.AluOpType.max)
            m_sb = mpool.tile([128, TILE_PX], f16, tag="m")
            if MINSUB == "a":
                # m' = relu(2 - d); colors carry +0.5
                nc.scalar.activation(m_sb[:], d_sb[:],
                                     mybir.ActivationFunctionType.Relu,
                                     bias=2.0, scale=-1.0)
            else:
                eng = nc.gpsimd if MINSUB == "g" else nc.vector
                eng.tensor_scalar(
                    m_sb[:], d_sb[:], 2.0, 2.0,
                    op0=mybir.AluOpType.min, op1=mybir.AluOpType.subtract)
            for rr in range(TILE_ROWS):
                lr = r * TILE_ROWS + rr
                off = 3 * (lr * N_BLOCKS + blk)
                nc.tensor.matmul(canvas[:, off:off + 3],
                                 m_sb[:, rr * TILE_COLS:(rr + 1) * TILE_COLS],
                                 CT_sb[:, 3 * ci:3 * ci + 3],
                                 start=(ci == first_chunk[t]),
                                 stop=(ci == last_chunk[t]))

        out_sb = opool.tile([128, ROWS_PER_CORE * N_BLOCKS * 3], f32)
        nc.scalar.activation(out_sb[:], canvas[:],
                             mybir.ActivationFunctionType.Sigmoid, scale=4.0)
        nc.sync.dma_start(y_d[:], out_sb[:])

    nc.compile()
    return nc


def _install_ntff_hook():
    """Provide antenv.axon_hooks if the image lacks it (ctypes shim around
    libaxon_pjrt.so's NRT profile capture). Returns True on success."""
    try:
        from antenv.axon_hooks import get_axon_ntff_profile_hook  # noqa: F401
        return True
    except ImportError:
        pass
    try:
        import contextlib
        import ctypes
        import types

        import antenv

        so_path = "/opt/axon/libaxon_pjrt.so"
        lib = ctypes.CDLL(so_path)
        if not hasattr(lib, "axon_start_nrt_profile"):
            return False
        lib.axon_start_nrt_profile.argtypes = [
            ctypes.POINTER(ctypes.c_int64), ctypes.c_size_t]
        lib.axon_start_nrt_profile.restype = ctypes.c_int64
        lib.axon_stop_nrt_profile.argtypes = [ctypes.c_char_p]
        lib.axon_stop_nrt_profile.restype = ctypes.c_int64

        @contextlib.contextmanager
        def _hook(output_dir, device_ids):
            import jax
            jax.devices()
            if device_ids:
                ids = (ctypes.c_int64 * len(device_ids))(*device_ids)
                rc = lib.axon_start_nrt_profile(ids, len(device_ids))
            else:
                rc = lib.axon_start_nrt_profile(None, 0)
            if rc != 0:
                raise RuntimeError(f"axon_start_nrt_profile rc={rc}")
            try:
                yield
            finally:
                n = lib.axon_stop_nrt_profile(str(output_dir).encode())
                print(f"ntff profile: {n} file(s) -> {output_dir}", file=sys.stderr)

        mod = types.ModuleType("antenv.axon_hooks")
        mod._hook = _hook
        mod.get_axon_ntff_profile_hook = lambda: _hook
        mod.set_axon_ntff_profile_hook = lambda h: None
        sys.modules["antenv.axon_hooks"] = mod
        antenv.axon_hooks = mod
        return True
    except Exception as e:  # pragma: no cover
        print("ntff hook install failed:", e, file=sys.stderr)
        return False


def kernel(locations, matrix_offsets, matrix_scale_exponents, colors,
           canvas_height_px, canvas_width_px):
    assert int(canvas_height_px) == H and int(canvas_width_px) == W

    w_np, g_np, ct_np, chunk_of_tile, n_chunk = _prepare(
        locations, matrix_offsets, matrix_scale_exponents, colors)

    nc = _build_nc(n_chunk, chunk_of_tile)

    from concourse.bass_utils import run_bass_kernel_spmd

    in_maps = [
        {"w": w_np[c], "g": g_np[c], "ct": ct_np[c]} for c in range(N_CORES)
    ]
    trace = bool(int(os.environ.get("BASS_KERNEL_TRACE", "0")))
    if trace:
        trace = _install_ntff_hook()
    try:
        res = run_bass_kernel_spmd(nc, in_maps, core_ids=list(range(N_CORES)),
                                   trace=trace)
    except Exception:
        if not trace:
            raise
        res = run_bass_kernel_spmd(nc, in_maps, core_ids=list(range(N_CORES)),
                                   trace=False)
    last_run_info.clear()
    last_run_info.update(
        exec_time_ns=res.exec_time_ns,
        mean_exec_time_ns=res.mean_exec_time_ns,
        profile_json=res.profile_json,
    )

    out = np.empty((3, H, W), np.float32)
    for c in range(N_CORES):
        y = res.results[c]["y"]                       # [128, 32*3*3]
        arr = y.reshape(128, ROWS_PER_CORE, N_BLOCKS, 3)  # p, lr, blk, ch
        out[:, c * ROWS_PER_CORE:(c + 1) * ROWS_PER_CORE, :] = (
            arr.transpose(3, 1, 2, 0).reshape(3, ROWS_PER_CORE, W))
    return out
